# revision 1
# baseline (speedup 1.0000x reference)
"""Trainium2 Bass kernel for DensityCalculator.

density[g] = sum_a sum_k aw[a,k]*exp(bw[a,k]*|g-x_a|^2) over a 64^3 grid,
then 3D FFT -> hamming filter -> inverse FFT -> normalize.

Distribution: grid x-axis sharded over 8 cores (8 x-planes each) for the
density phase; FFT is a distributed pencil decomposition (z,y transforms
local to the x-slab, AllToAll to kz-slabs, x transform + filter + inverse x,
AllToAll back, inverse y,z local).

Device algebra:
 - d2 via TensorE: [gx,gy,gz,1,|g|^2] . [-2X; |X|^2; 1]  (K=5 matmul, f32r)
 - aw*exp(bw*d2) in ONE scalar-engine op per k: Exp with per-partition
   scale=bw[:,k], bias=ln(aw[:,k])  (requires aw >= 0, true for this model)
 - sum over atoms via ones-vector matmul accumulated over k in PSUM
 - FFT as matmuls with 64x64 DFT matrices (fftshift/ifftshift folded into
   column/row permutations host-side), transposes on TensorE.
"""

import os
import sys
import numpy as np

for _p in ("/opt/trn_rl_repo", "/root/.axon_site", "/root/.axon_site/_ro/trn_rl_repo",
           "/root/.axon_site/_ro/pypackages"):
    if _p not in sys.path and os.path.isdir(_p):
        sys.path.append(_p)

import concourse.bass as bass
import concourse.tile as tile
from concourse import bacc, mybir
from concourse.bass_utils import run_bass_kernel_spmd

FP = mybir.dt.float32
FR = mybir.dt.float32r
BF = mybir.dt.bfloat16
Exp = mybir.ActivationFunctionType.Exp

N_CORES = 8
N = 64              # grid size per axis
A = 128             # atoms
K = 6               # gaussian terms
XL = 8              # x-planes per core
GPC = N * XL * N    # grid points per core (32768)
NG = 8              # phase-1 groups per core
GSZ = GPC // NG     # 4096 points per group

LAST_EXEC_NS = None
LAST_RESULTS = None
_COMPILED = None
DEBUG_TAPS = bool(os.environ.get("KERNEL_DEBUG_TAPS"))


def _build():
    nc = bacc.Bacc("TRN2", target_bir_lowering=False, debug=False,
                   num_devices=N_CORES)

    rhs5 = nc.dram_tensor("rhs5", [5, GPC], FP, kind="ExternalInput").ap()
    xstat = nc.dram_tensor("xstat", [5, A], FP, kind="ExternalInput").ap()
    bwln = nc.dram_tensor("bwln", [A, 2 * K], FP, kind="ExternalInput").ap()
    mats = nc.dram_tensor("mats", [N, 128 + 7 * N], FR, kind="ExternalInput").ap()
    ham = nc.dram_tensor("ham", [N, 512], FR, kind="ExternalInput").ap()
    mats2 = nc.dram_tensor("mats2", [8, 8 * 128], FR, kind="ExternalInput").ap()
    out = nc.dram_tensor("out", [N, 512], FP, kind="ExternalOutput").ap()
    taps = {}
    if DEBUG_TAPS:
        for nm in ("dbg_rho", "dbg_f1re", "dbg_f1im", "dbg_t1re", "dbg_t1im",
                   "dbg_f2re", "dbg_f2im", "dbg_g2re", "dbg_g2im",
                   "dbg_t2re", "dbg_t2im", "dbg_f3re", "dbg_f3im",
                   "dbg_g5re", "dbg_g5im"):
            taps[nm] = nc.dram_tensor(nm, [N, 512], FR, kind="ExternalOutput").ap()

    def tap(nm, tile_):
        if DEBUG_TAPS:
            nc.sync.dma_start(taps[nm], tile_[:])

    with tile.TileContext(nc) as tc:
        with tc.tile_pool(name="const", bufs=1) as constp, \
             tc.tile_pool(name="dram", bufs=1, space="DRAM") as dram:
            xstat_sb = constp.tile([5, A], FP)
            nc.sync.dma_start(xstat_sb[:], xstat[:])
            bwln_sb = constp.tile([A, 2 * K], FP)
            nc.sync.dma_start(bwln_sb[:], bwln[:])
            mats_sb = constp.tile([N, 128 + 7 * N], FR)
            nc.sync.dma_start(mats_sb[:], mats[:])
            ham_sb = constp.tile([N, 512], FR)
            nc.sync.dma_start(ham_sb[:], ham[:])
            mats2_sb = constp.tile([8, 8 * 128], FR)
            nc.sync.dma_start(mats2_sb[:], mats2[:])
            # e8[:, j*8:(j+1)*8] is a (128,8) stationary whose col j is all-ones:
            # chunk j's atom-sum matmul lands on psum partition j.
            e8 = constp.tile([A, 64], BF)
            nc.vector.memset(e8[:], 0.0)
            for j in range(8):
                nc.vector.memset(e8[:, j * 8 + j:j * 8 + j + 1], 1.0)

            # tiny warmup AllToAll so ncfw channel setup overlaps phase 1
            wu_in = dram.tile([N_CORES, 8], FP, tag="wu_in")
            wu_out = dram.tile([N_CORES, 8], FP, tag="wu_out")
            wu_sb = constp.tile([1, N_CORES * 8], FP)
            nc.vector.memset(wu_sb[:], 0.0)
            nc.sync.dma_start(wu_in[:].rearrange("a b -> (a b)"), wu_sb[0, :])
            nc.gpsimd.collective_compute(
                "AllToAll", mybir.AluOpType.bypass,
                replica_groups=[list(range(N_CORES))],
                ins=[wu_in.opt()], outs=[wu_out.opt()])

            # stationary views into mats
            Az2T = mats_sb[:, 0:128]
            c0 = 128
            ArT = mats_sb[:, c0:c0 + N]
            AiT = mats_sb[:, c0 + N:c0 + 2 * N]
            AiTn = mats_sb[:, c0 + 2 * N:c0 + 3 * N]
            BrT = mats_sb[:, c0 + 3 * N:c0 + 4 * N]
            BiT = mats_sb[:, c0 + 4 * N:c0 + 5 * N]
            BiTn = mats_sb[:, c0 + 5 * N:c0 + 6 * N]
            ident = mats_sb[:, c0 + 6 * N:c0 + 7 * N]

            # ---------------- Phase 1: density ----------------
            acc_pool = tc.tile_pool(name="p1acc", bufs=1, space="PSUM")
            accps = acc_pool.__enter__().tile([128, 512], FP, tag="acc")
            with tc.tile_pool(name="p1sb", bufs=2) as p1sb, \
                 tc.tile_pool(name="p1e", bufs=1) as p1e, \
                 tc.tile_pool(name="p1ps", bufs=1, space="PSUM") as p1ps, \
                 tc.tile_pool(name="p1rho", bufs=3, space="PSUM") as p1rho:
                def emit_d2(g):
                    """PE: 8 K=5 fp32 matmuls -> PSUM; DVE: copy to SBUF."""
                    rh = p1sb.tile([5, GSZ], FP, tag="rh")
                    nc.sync.dma_start(rh[:], rhs5[:, g * GSZ:(g + 1) * GSZ])
                    d2sb = p1sb.tile([A, GSZ], FP, tag="d2")
                    for h in range(2):
                        d2ps = p1ps.tile([A, 2048], FP, tag="d2ps")
                        for j in range(4):
                            nc.tensor.matmul(
                                d2ps[:, j * 512:(j + 1) * 512],
                                lhsT=xstat_sb[:],
                                rhs=rh[:, h * 2048 + j * 512:h * 2048 + (j + 1) * 512],
                                start=True, stop=True)
                        nc.vector.tensor_copy(d2sb[:, h * 2048:(h + 1) * 2048], d2ps[:])
                    return d2sb

                # software pipeline: emit group g+1's d2 matmuls BEFORE group
                # g's atom-sum matmuls so the PE queue never stalls on scalar.
                d2_next = emit_d2(0)
                for g in range(NG):
                    d2sb = d2_next
                    Es = []
                    for k in range(K):
                        Ek = p1e.tile([A, GSZ], BF, tag=f"E{k}")
                        if g == 0:
                            # split so scalar starts on the first PSUM half sooner
                            for h in range(2):
                                nc.scalar.activation(Ek[:, h * 2048:(h + 1) * 2048],
                                                     d2sb[:, h * 2048:(h + 1) * 2048], Exp,
                                                     bias=bwln_sb[:, K + k:K + k + 1],
                                                     scale=bwln_sb[:, k:k + 1])
                        else:
                            nc.scalar.activation(Ek[:], d2sb[:], Exp,
                                                 bias=bwln_sb[:, K + k:K + k + 1],
                                                 scale=bwln_sb[:, k:k + 1])
                        Es.append(Ek)
                    if g + 1 < NG:
                        d2_next = emit_d2(g + 1)
                    # pair-sum the six E_k on DVE (bf16) to cut atom-sum matmuls 2x
                    Ps = []
                    for i in range(3):
                        Pi = p1e.tile([A, GSZ], BF, tag=f"P{i}")
                        nc.vector.tensor_tensor(Pi[:], Es[2 * i][:], Es[2 * i + 1][:],
                                                op=mybir.AluOpType.add)
                        Ps.append(Pi)
                    rps = p1rho.tile([8, 512], FP, tag="rps")
                    for j in range(NG):
                        for i in range(3):
                            nc.tensor.matmul(rps[:],
                                             lhsT=e8[:, j * 8:(j + 1) * 8],
                                             rhs=Ps[i][:, j * 512:(j + 1) * 512],
                                             start=(j == 0 and i == 0),
                                             stop=(j == NG - 1 and i == 2))
                    stage8 = p1sb.tile([8, 512], FR, tag="st8")
                    nc.vector.tensor_copy(stage8[:], rps[:])
                    # incremental forward-z transform: acc += Az2T[8g:8g+8].T @ stage8
                    nc.tensor.matmul(accps[:], lhsT=mats2_sb[:, g * 128:(g + 1) * 128],
                                     rhs=stage8[:], start=(g == 0), stop=(g == NG - 1))

            # ---------------- Phase 2: distributed FFT ----------------
            def cpass(fsb, fpsp, sre, sim_n, sim_, re_in, im_in, stacked=False):
                """complex pass: re_out = sre@re + sim_n@im ; im_out = sim_@re + sre@im
                (sim_n = negated imag matrix). Round-copies to f32r tiles.
                stacked=True returns one (128,512) tile [re | im] for A2A staging."""
                ps_re = fpsp.tile([N, 512], FP, tag="psre")
                ps_im = fpsp.tile([N, 512], FP, tag="psim")
                nc.tensor.matmul(ps_re[:], lhsT=sre, rhs=re_in[:], start=True, stop=False)
                nc.tensor.matmul(ps_re[:], lhsT=sim_n, rhs=im_in[:], start=False, stop=True)
                nc.tensor.matmul(ps_im[:], lhsT=sim_, rhs=re_in[:], start=True, stop=False)
                nc.tensor.matmul(ps_im[:], lhsT=sre, rhs=im_in[:], start=False, stop=True)
                if stacked:
                    o = fsb.tile([128, 512], FR, tag="fstk")
                    nc.vector.tensor_copy(o[0:N, :], ps_re[:])
                    nc.vector.tensor_copy(o[N:128, :], ps_im[:])
                    return o
                o_re = fsb.tile([N, 512], FR, tag="fre")
                o_im = fsb.tile([N, 512], FR, tag="fim")
                nc.vector.tensor_copy(o_re[:], ps_re[:])
                nc.vector.tensor_copy(o_im[:], ps_im[:])
                return o_re, o_im

            dma_engines = (nc.sync, nc.scalar, nc.gpsimd, nc.sync)

            def tstage(fsb, tps, re_in, im_in, strided_out):
                """transpose stage. Input [p | blk*64+q] (blk outer, q inner 64).
                strided_out=False: out[q | blk*64+p]  (contiguous 64-col writes)
                strided_out=True:  out[q | p*8+blk]   (stride-8 writes, 8 blocks)
                """
                o_re = fsb.tile([N, 512], FR, tag="tre")
                o_im = fsb.tile([N, 512], FR, tag="tim")
                for (src, dst) in ((re_in, o_re), (im_in, o_im)):
                    dstv = dst.rearrange("p (q b) -> p q b", q=N, b=8) if strided_out else None
                    for b in range(4):
                        pt = tps.tile([128, N], FR, tag="pt")
                        nc.tensor.transpose(pt[:], src[:, b * 128:(b + 1) * 128], ident)
                        for bb in range(2):
                            blk = 2 * b + bb
                            if strided_out:
                                nc.vector.tensor_copy(dstv[:, :, blk],
                                                      pt[bb * N:(bb + 1) * N, :])
                            else:
                                nc.vector.tensor_copy(dst[:, blk * N:(blk + 1) * N],
                                                      pt[bb * N:(bb + 1) * N, :])
                return o_re, o_im

            fsb_pool = tc.tile_pool(name="fft", bufs=2)
            fsb = fsb_pool.__enter__()
            # forward z was accumulated during phase 1; just round-copy out
            f_re = fsb.tile([N, 512], FR, tag="fre")
            f_im = fsb.tile([N, 512], FR, tag="fim")
            nc.vector.tensor_copy(f_re[:], accps[0:N, :])
            nc.vector.tensor_copy(f_im[:], accps[N:128, :])
            acc_pool.__exit__(None, None, None)
            with tc.tile_pool(name="fps", bufs=2, space="PSUM") as fps, \
                 tc.tile_pool(name="ps6p", bufs=1, space="PSUM") as ps6p, \
                 tc.tile_pool(name="tps", bufs=3, space="PSUM") as tps:
                tap("dbg_f1re", f_re); tap("dbg_f1im", f_im)
                t_re, t_im = tstage(fsb, tps, f_re, f_im, True)   # [y | kz*8+xl]
                tap("dbg_t1re", t_re); tap("dbg_t1im", t_im)
                f2_re, f2_im = cpass(fsb, fps, ArT, AiTn, AiT, t_re, t_im)  # [ky | kz*8+xl]

                tap("dbg_f2re", f2_re); tap("dbg_f2im", f2_im)
                # A2A #1: -> [ky | kzl*64 + x]   (chunk for dest d = cols [64d,64d+64))
                a_in = dram.tile([N_CORES, 2, N, 8, 8], FR, tag="a2a_in")
                a_out = dram.tile([N_CORES, 2, N, 8, 8], FR, tag="a2a_out")
                for dd in range(N_CORES):
                    nc.sync.dma_start(a_in[dd, 0], f2_re[:, dd * N:(dd + 1) * N])
                    nc.sync.dma_start(a_in[dd, 1], f2_im[:, dd * N:(dd + 1) * N])
                nc.gpsimd.collective_compute(
                    "AllToAll", mybir.AluOpType.bypass,
                    replica_groups=[list(range(N_CORES))],
                    ins=[a_in.opt()], outs=[a_out.opt()])
                # recv contiguously (cheap descriptors), then DVE-permute
                # [p | s*64+kl*8+xl] -> [p | kl*64+s*8+xl]
                g_rre = fsb.tile([N, 512], FR, tag="grre")
                g_rim = fsb.tile([N, 512], FR, tag="grim")
                for ss in range(N_CORES):
                    nc.sync.dma_start(
                        g_rre[:, ss * N:(ss + 1) * N].rearrange("p (kl xl) -> p kl xl", kl=8, xl=8),
                        a_out[ss, 0])
                    nc.sync.dma_start(
                        g_rim[:, ss * N:(ss + 1) * N].rearrange("p (kl xl) -> p kl xl", kl=8, xl=8),
                        a_out[ss, 1])
                g_re = fsb.tile([N, 512], FR, tag="fre")
                g_im = fsb.tile([N, 512], FR, tag="fim")
                nc.vector.tensor_copy(
                    g_re.rearrange("p (kl s xl) -> p s kl xl", kl=8, s=8, xl=8),
                    g_rre.rearrange("p (s kl xl) -> p s kl xl", s=8, kl=8, xl=8))
                nc.vector.tensor_copy(
                    g_im.rearrange("p (kl s xl) -> p s kl xl", kl=8, s=8, xl=8),
                    g_rim.rearrange("p (s kl xl) -> p s kl xl", s=8, kl=8, xl=8))

                tap("dbg_g2re", g_re); tap("dbg_g2im", g_im)
                t2_re, t2_im = tstage(fsb, tps, g_re, g_im, False)  # [x | kzl*64+ky]
                tap("dbg_t2re", t2_re); tap("dbg_t2im", t2_im)
                # P3 forward x, then filter fused into the PSUM->SBUF copy
                ps3_re = fps.tile([N, 512], FP, tag="psre")
                ps3_im = fps.tile([N, 512], FP, tag="psim")
                nc.tensor.matmul(ps3_re[:], lhsT=ArT, rhs=t2_re[:], start=True, stop=False)
                nc.tensor.matmul(ps3_re[:], lhsT=AiTn, rhs=t2_im[:], start=False, stop=True)
                nc.tensor.matmul(ps3_im[:], lhsT=AiT, rhs=t2_re[:], start=True, stop=False)
                nc.tensor.matmul(ps3_im[:], lhsT=ArT, rhs=t2_im[:], start=False, stop=True)
                f3_re = fsb.tile([N, 512], FR, tag="fre")
                f3_im = fsb.tile([N, 512], FR, tag="fim")
                nc.vector.tensor_tensor(f3_re[:], ps3_re[:], ham_sb[:], op=mybir.AluOpType.mult)
                nc.vector.tensor_tensor(f3_im[:], ps3_im[:], ham_sb[:], op=mybir.AluOpType.mult)

                tap("dbg_f3re", f3_re); tap("dbg_f3im", f3_im)
                g4_re, g4_im = cpass(fsb, fps, BrT, BiTn, BiT, f3_re, f3_im)  # [x | kzl*64+ky]
                t3_re, t3_im = tstage(fsb, tps, g4_re, g4_im, True)  # [ky | x*8+kzl]
                f5s = cpass(fsb, fps, BrT, BiTn, BiT, t3_re, t3_im, stacked=True)  # [y± | x*8+kzl]

                # A2A #2: -> [y | xl*64 + kz]   (chunk for dest d = cols [64d,64d+64))
                a2_in = dram.tile([N_CORES, 2, N, 8, 8], FR, tag="a2a2_in")
                a2_out = dram.tile([N_CORES, 2, N, 8, 8], FR, tag="a2a2_out")
                for dd in range(N_CORES):
                    dma_engines[dd % 4].dma_start(
                        a2_in[dd].rearrange("q p xl kl -> (q p) xl kl"),
                        f5s[:, dd * N:(dd + 1) * N].rearrange("p (xl kl) -> p xl kl", xl=8, kl=8))
                nc.gpsimd.collective_compute(
                    "AllToAll", mybir.AluOpType.bypass,
                    replica_groups=[list(range(N_CORES))],
                    ins=[a2_in.opt()], outs=[a2_out.opt()])
                # recv contiguously, then DVE-permute [p | s*64+xl*8+kl] -> [p | xl*64+s*8+kl]
                g5_raw = fsb.tile([128, 512], FR, tag="graw")
                for ss in range(N_CORES):
                    dma_engines[ss % 4].dma_start(
                        g5_raw[:, ss * N:(ss + 1) * N].rearrange("p (xl kl) -> p xl kl", xl=8, kl=8),
                        a2_out[ss].rearrange("q p xl kl -> (q p) xl kl"))
                g5_re = fsb.tile([N, 512], FR, tag="fre")
                g5_im = fsb.tile([N, 512], FR, tag="fim")
                nc.vector.tensor_copy(
                    g5_re.rearrange("p (xl s kl) -> p s xl kl", xl=8, s=8, kl=8),
                    g5_raw[0:N, :].rearrange("p (s xl kl) -> p s xl kl", s=8, xl=8, kl=8))
                nc.vector.tensor_copy(
                    g5_im.rearrange("p (xl s kl) -> p s xl kl", xl=8, s=8, kl=8),
                    g5_raw[N:128, :].rearrange("p (s xl kl) -> p s xl kl", s=8, xl=8, kl=8))

                tap("dbg_g5re", g5_re); tap("dbg_g5im", g5_im)
                t4_re, t4_im = tstage(fsb, tps, g5_re, g5_im, False)  # [kz | xl*64+y]
                # P6: inverse z, real part only
                ps6 = ps6p.tile([N, 512], FP, tag="ps6")
                nc.tensor.matmul(ps6[:], lhsT=BrT, rhs=t4_re[:], start=True, stop=False)
                nc.tensor.matmul(ps6[:], lhsT=BiTn, rhs=t4_im[:], start=False, stop=True)
                out_sb = fsb.tile([N, 512], FP, tag="osb")
                nc.vector.tensor_copy(out_sb[:], ps6[:])
                nc.sync.dma_start(out[:], out_sb[:])
            fsb_pool.__exit__(None, None, None)

    nc.compile()
    return nc


def _get_compiled():
    global _COMPILED
    if _COMPILED is None:
        _COMPILED = _build()
    return _COMPILED


def _host_inputs(X, aw, bw, real_grid_flat, hamming):
    X = np.asarray(X, np.float32)
    aw = np.asarray(aw, np.float32)
    bw = np.asarray(bw, np.float32)
    grid = np.asarray(real_grid_flat, np.float32)
    hamming = np.asarray(hamming, np.float32)

    arr = grid.reshape(N, N, N, 3)                       # [x, y, z, 3]
    arrt = np.transpose(arr, (2, 0, 1, 3))               # [z, x, y, 3]
    g2 = (arrt.astype(np.float64) ** 2).sum(-1).astype(np.float32)
    rhs5_full = np.stack(
        [arrt[..., 0], arrt[..., 1], arrt[..., 2],
         np.ones((N, N, N), np.float32), g2], 0)          # (5, z, x, y)

    xstat = np.concatenate(
        [-2.0 * X.T, (X.astype(np.float64) ** 2).sum(-1)[None, :].astype(np.float32),
         np.ones((1, A), np.float32)], 0).astype(np.float32)   # (5, 128)

    lnaw = np.log(np.maximum(aw, 1e-38)).astype(np.float32)
    bwln = np.concatenate([bw, lnaw], 1).astype(np.float32)     # (128, 12)

    F = np.fft.fft(np.eye(N), axis=0, norm='ortho')
    IF = np.fft.ifft(np.eye(N), axis=0, norm='ortho')
    perm = (np.arange(N) + N // 2) % N
    Am = F[:, perm]
    Bm = IF[perm, :]
    Ar, Ai = Am.real.astype(np.float32), Am.imag.astype(np.float32)
    Br, Bi = Bm.real.astype(np.float32), Bm.imag.astype(np.float32)
    Az2T = np.concatenate([Ar.T, Ai.T], 1)               # (64, 128)
    mats2 = np.ascontiguousarray(
        np.transpose(Az2T.reshape(8, 8, 128), (1, 0, 2))).reshape(8, 8 * 128)
    mats = np.concatenate(
        [Az2T, Ar.T, Ai.T, -Ai.T, Br.T, Bi.T, -Bi.T,
         np.eye(N, dtype=np.float32)], 1)                # (64, 576)

    Hfull = np.fft.ifftshift(hamming)                    # [kx, ky, kz]

    in_maps = []
    for c in range(N_CORES):
        rhs5c = np.ascontiguousarray(
            rhs5_full[:, :, 8 * c:8 * (c + 1), :]).reshape(5, GPC)
        Hc = np.ascontiguousarray(
            np.transpose(Hfull[:, :, 8 * c:8 * (c + 1)], (0, 2, 1))).reshape(N, 512)
        in_maps.append({"rhs5": rhs5c, "xstat": xstat, "bwln": bwln,
                        "mats": mats, "mats2": mats2, "ham": Hc})
    return in_maps


def kernel(X, aw, bw, real_grid_flat, hamming):
    global LAST_EXEC_NS, LAST_RESULTS
    in_maps = _host_inputs(X, aw, bw, real_grid_flat, hamming)
    nc = _get_compiled()

    trace = bool(os.environ.get("BASS_TRACE"))
    res = run_bass_kernel_spmd(nc, in_maps, core_ids=list(range(N_CORES)),
                               trace=trace)
    LAST_EXEC_NS = res.exec_time_ns
    global LAST_RESULTS
    LAST_RESULTS = res.results

    full = np.empty((N, N, N), np.float32)               # [z, x, y]
    for c in range(N_CORES):
        full[:, 8 * c:8 * (c + 1), :] = res.results[c]["out"].reshape(N, 8, N)
    o = np.transpose(full, (1, 2, 0))                    # [x, y, z]
    o = (o - o.mean()) / (o.std() + 1e-8)
    return o.astype(np.float32)



# revision 8
# speedup vs baseline: 2.1715x; 2.1715x over previous
"""Trainium2 Bass kernel for DensityCalculator.

density[g] = sum_a sum_k aw[a,k]*exp(bw[a,k]*|g-x_a|^2) over a 64^3 grid,
then 3D FFT -> hamming filter -> inverse FFT -> normalize.

Distribution: grid x-axis sharded over 8 cores (8 x-planes each) for the
density phase; FFT is a distributed pencil decomposition (z,y transforms
local to the x-slab, AllToAll to kz-slabs, x transform + filter + inverse x,
AllToAll back, inverse y,z local).

Device algebra (separable Gaussian splatting):
 - the Gaussian factors per axis: aw*exp(bw*|g-x|^2) =
   prod_axis exp(bw*d_axis^2 + ln(aw)/3), so only 136 distinct 1-D
   coordinate offsets per core (8 slab-x + 64 y + 64 z) need exps:
   6 scalar-engine Exp ops over [128 atoms, 136] with per-partition
   scale=bw[:,k], bias=ln(aw[:,k])/3
 - G_k[a, (x,y)] = Ex_k (x) Ey_k outer product via one DVE broadcast
   multiply per k (bf16)
 - rho[z, (x,y)] = sum_k sum_a Ez_k[a,z] * G_k[a,(x,y)]: 6 accumulating
   bf16 matmuls with Ez_k as lhsT (atom contraction on the PE)
 - FFT as matmuls with 64x64 DFT matrices (fftshift/ifftshift folded into
   column/row permutations host-side), transposes on TensorE.
"""

import os
import sys
import numpy as np

for _p in ("/opt/trn_rl_repo", "/root/.axon_site", "/root/.axon_site/_ro/trn_rl_repo",
           "/root/.axon_site/_ro/pypackages"):
    if _p not in sys.path and os.path.isdir(_p):
        sys.path.append(_p)

import concourse.bass as bass
import concourse.tile as tile
from concourse import bacc, mybir
from concourse.bass_utils import run_bass_kernel_spmd

FP = mybir.dt.float32
FR = mybir.dt.float32r
BF = mybir.dt.bfloat16
Exp = mybir.ActivationFunctionType.Exp

N_CORES = 8
N = 64              # grid size per axis
A = 128             # atoms
K = 6               # gaussian terms
XL = 8              # x-planes per core
GPC = N * XL * N    # grid points per core (32768)
D1 = XL + 2 * N     # 1-D separable offsets per core (136: x-slab, y, z)

LAST_EXEC_NS = None
LAST_RESULTS = None
_COMPILED = None
DEBUG_TAPS = bool(os.environ.get("KERNEL_DEBUG_TAPS"))


def _build():
    nc = bacc.Bacc("TRN2", target_bir_lowering=False, debug=False,
                   num_devices=N_CORES)

    d2all = nc.dram_tensor("d2all", [A, D1], FP, kind="ExternalInput").ap()
    bwln = nc.dram_tensor("bwln", [A, 2 * K], FP, kind="ExternalInput").ap()
    mats = nc.dram_tensor("mats", [N, 128 + 7 * N], FR, kind="ExternalInput").ap()
    ham = nc.dram_tensor("ham", [N, 512], FR, kind="ExternalInput").ap()
    out = nc.dram_tensor("out", [N, 512], FP, kind="ExternalOutput").ap()
    taps = {}
    if DEBUG_TAPS:
        for nm in ("dbg_rho", "dbg_f1re", "dbg_f1im", "dbg_t1re", "dbg_t1im",
                   "dbg_f2re", "dbg_f2im", "dbg_g2re", "dbg_g2im",
                   "dbg_t2re", "dbg_t2im", "dbg_f3re", "dbg_f3im",
                   "dbg_g5re", "dbg_g5im"):
            taps[nm] = nc.dram_tensor(nm, [N, 512], FR, kind="ExternalOutput").ap()

    def tap(nm, tile_):
        if DEBUG_TAPS:
            nc.sync.dma_start(taps[nm], tile_[:])

    with tile.TileContext(nc) as tc:
        with tc.tile_pool(name="const", bufs=1) as constp, \
             tc.tile_pool(name="dram", bufs=1, space="DRAM") as dram:
            d2_sb = constp.tile([A, D1], FP)
            nc.sync.dma_start(d2_sb[:], d2all[:])
            bwln_sb = constp.tile([A, 2 * K], FP)
            nc.sync.dma_start(bwln_sb[:], bwln[:])
            mats_sb = constp.tile([N, 128 + 7 * N], FR)
            nc.scalar.dma_start(mats_sb[:], mats[:])
            ham_sb = constp.tile([N, 512], FR)
            nc.scalar.dma_start(ham_sb[:], ham[:])

            # tiny warmup AllToAll so ncfw channel setup overlaps phase 1
            wu_in = dram.tile([N_CORES, 8], FP, tag="wu_in")
            wu_out = dram.tile([N_CORES, 8], FP, tag="wu_out")
            wu_sb = constp.tile([1, N_CORES * 8], FP)
            nc.vector.memset(wu_sb[:], 0.0)
            nc.sync.dma_start(wu_in[:].rearrange("a b -> (a b)"), wu_sb[0, :])
            nc.gpsimd.collective_compute(
                "AllToAll", mybir.AluOpType.bypass,
                replica_groups=[list(range(N_CORES))],
                ins=[wu_in.opt()], outs=[wu_out.opt()])

            # stationary views into mats
            Az2T = mats_sb[:, 0:128]
            c0 = 128
            ArT = mats_sb[:, c0:c0 + N]
            AiT = mats_sb[:, c0 + N:c0 + 2 * N]
            AiTn = mats_sb[:, c0 + 2 * N:c0 + 3 * N]
            BrT = mats_sb[:, c0 + 3 * N:c0 + 4 * N]
            BiT = mats_sb[:, c0 + 4 * N:c0 + 5 * N]
            BiTn = mats_sb[:, c0 + 5 * N:c0 + 6 * N]
            ident = mats_sb[:, c0 + 6 * N:c0 + 7 * N]

            # ---------------- Phase 1: separable density ----------------
            acc_pool = tc.tile_pool(name="p1acc", bufs=1, space="PSUM")
            accps = acc_pool.__enter__().tile([128, 512], FP, tag="acc")
            with tc.tile_pool(name="p1sb", bufs=1) as p1sb, \
                 tc.tile_pool(name="p1rho", bufs=1, space="PSUM") as p1rho:
                # E[a, k, :] = exp(bw_k*d2 + ln(aw_k)/3) over the 136 1-D
                # offsets; each of the three axis factors carries aw^(1/3)
                # so their product carries aw exactly once.
                E = p1sb.tile([A, K, D1], BF, tag="E")
                for k in range(K):
                    nc.scalar.activation(E[:, k, :], d2_sb[:], Exp,
                                         bias=bwln_sb[:, K + k:K + k + 1],
                                         scale=bwln_sb[:, k:k + 1])
                # G_k[a, x, y] = Ex_k (x) Ey_k  (DVE stride-0 broadcast mult)
                G = p1sb.tile([A, K, XL, N], BF, tag="G")
                for k in range(K):
                    nc.vector.tensor_tensor(
                        G[:, k],
                        E[:, k, 0:XL][:, :, None].broadcast_to([A, XL, N]),
                        E[:, k, XL:XL + N][:, None, :].broadcast_to([A, XL, N]),
                        op=mybir.AluOpType.mult)
                # rho[z, (x,y)] = sum_k Ez_k^T @ G_k  (atom contraction)
                rho_ps = p1rho.tile([N, 512], FP, tag="rho")
                for k in range(K):
                    nc.tensor.matmul(rho_ps[:],
                                     lhsT=E[:, k, XL + N:XL + 2 * N],
                                     rhs=G[:, k].rearrange("p x y -> p (x y)"),
                                     start=(k == 0), stop=(k == K - 1))
                rho_sb = p1sb.tile([N, 512], FR, tag="rho_sb")
                nc.vector.tensor_copy(rho_sb[:], rho_ps[:])
                tap("dbg_rho", rho_sb)
                # forward z-FFT: accps[(kz_re|kz_im), (x,y)] = Az2T^T @ rho
                nc.tensor.matmul(accps[:], lhsT=Az2T, rhs=rho_sb[:],
                                 start=True, stop=True)

            # ---------------- Phase 2: distributed FFT ----------------
            def cpass(fsb, fpsp, sre, sim_n, sim_, re_in, im_in, stacked=False):
                """complex pass: re_out = sre@re + sim_n@im ; im_out = sim_@re + sre@im
                (sim_n = negated imag matrix). Round-copies to f32r tiles.
                stacked=True returns one (128,512) tile [re | im] for A2A staging."""
                ps_re = fpsp.tile([N, 512], FP, tag="psre")
                ps_im = fpsp.tile([N, 512], FP, tag="psim")
                nc.tensor.matmul(ps_re[:], lhsT=sre, rhs=re_in[:], start=True, stop=False)
                nc.tensor.matmul(ps_re[:], lhsT=sim_n, rhs=im_in[:], start=False, stop=True)
                nc.tensor.matmul(ps_im[:], lhsT=sim_, rhs=re_in[:], start=True, stop=False)
                nc.tensor.matmul(ps_im[:], lhsT=sre, rhs=im_in[:], start=False, stop=True)
                if stacked:
                    o = fsb.tile([128, 512], FR, tag="fstk")
                    nc.vector.tensor_copy(o[0:N, :], ps_re[:])
                    nc.vector.tensor_copy(o[N:128, :], ps_im[:])
                    return o
                o_re = fsb.tile([N, 512], FR, tag="fre")
                o_im = fsb.tile([N, 512], FR, tag="fim")
                nc.vector.tensor_copy(o_re[:], ps_re[:])
                nc.vector.tensor_copy(o_im[:], ps_im[:])
                return o_re, o_im

            dma_engines = (nc.sync, nc.scalar, nc.gpsimd, nc.sync)

            def tstage(fsb, tps, re_in, im_in, strided_out):
                """transpose stage. Input [p | blk*64+q] (blk outer, q inner 64).
                strided_out=False: out[q | blk*64+p]  (contiguous 64-col writes)
                strided_out=True:  out[q | p*8+blk]   (stride-8 writes, 8 blocks)
                """
                o_re = fsb.tile([N, 512], FR, tag="tre")
                o_im = fsb.tile([N, 512], FR, tag="tim")
                for (src, dst) in ((re_in, o_re), (im_in, o_im)):
                    dstv = dst.rearrange("p (q b) -> p q b", q=N, b=8) if strided_out else None
                    for b in range(4):
                        pt = tps.tile([128, N], FR, tag="pt")
                        nc.tensor.transpose(pt[:], src[:, b * 128:(b + 1) * 128], ident)
                        for bb in range(2):
                            blk = 2 * b + bb
                            if strided_out:
                                nc.vector.tensor_copy(dstv[:, :, blk],
                                                      pt[bb * N:(bb + 1) * N, :])
                            else:
                                nc.vector.tensor_copy(dst[:, blk * N:(blk + 1) * N],
                                                      pt[bb * N:(bb + 1) * N, :])
                return o_re, o_im

            fsb_pool = tc.tile_pool(name="fft", bufs=2)
            fsb = fsb_pool.__enter__()
            # forward z was accumulated during phase 1; just round-copy out
            f_re = fsb.tile([N, 512], FR, tag="fre")
            f_im = fsb.tile([N, 512], FR, tag="fim")
            nc.vector.tensor_copy(f_re[:], accps[0:N, :])
            nc.vector.tensor_copy(f_im[:], accps[N:128, :])
            acc_pool.__exit__(None, None, None)
            with tc.tile_pool(name="fps", bufs=2, space="PSUM") as fps, \
                 tc.tile_pool(name="ps6p", bufs=1, space="PSUM") as ps6p, \
                 tc.tile_pool(name="tps", bufs=3, space="PSUM") as tps:
                tap("dbg_f1re", f_re); tap("dbg_f1im", f_im)
                t_re, t_im = tstage(fsb, tps, f_re, f_im, True)   # [y | kz*8+xl]
                tap("dbg_t1re", t_re); tap("dbg_t1im", t_im)
                f2_re, f2_im = cpass(fsb, fps, ArT, AiTn, AiT, t_re, t_im)  # [ky | kz*8+xl]

                tap("dbg_f2re", f2_re); tap("dbg_f2im", f2_im)
                # A2A #1: -> [ky | kzl*64 + x]   (chunk for dest d = cols [64d,64d+64))
                a_in = dram.tile([N_CORES, 2, N, 8, 8], FR, tag="a2a_in")
                a_out = dram.tile([N_CORES, 2, N, 8, 8], FR, tag="a2a_out")
                for dd in range(N_CORES):
                    nc.sync.dma_start(a_in[dd, 0], f2_re[:, dd * N:(dd + 1) * N])
                    nc.sync.dma_start(a_in[dd, 1], f2_im[:, dd * N:(dd + 1) * N])
                nc.gpsimd.collective_compute(
                    "AllToAll", mybir.AluOpType.bypass,
                    replica_groups=[list(range(N_CORES))],
                    ins=[a_in.opt()], outs=[a_out.opt()])
                # recv contiguously (cheap descriptors), then DVE-permute
                # [p | s*64+kl*8+xl] -> [p | kl*64+s*8+xl]
                g_rre = fsb.tile([N, 512], FR, tag="grre")
                g_rim = fsb.tile([N, 512], FR, tag="grim")
                for ss in range(N_CORES):
                    nc.sync.dma_start(
                        g_rre[:, ss * N:(ss + 1) * N].rearrange("p (kl xl) -> p kl xl", kl=8, xl=8),
                        a_out[ss, 0])
                    nc.sync.dma_start(
                        g_rim[:, ss * N:(ss + 1) * N].rearrange("p (kl xl) -> p kl xl", kl=8, xl=8),
                        a_out[ss, 1])
                g_re = fsb.tile([N, 512], FR, tag="fre")
                g_im = fsb.tile([N, 512], FR, tag="fim")
                nc.vector.tensor_copy(
                    g_re.rearrange("p (kl s xl) -> p s kl xl", kl=8, s=8, xl=8),
                    g_rre.rearrange("p (s kl xl) -> p s kl xl", s=8, kl=8, xl=8))
                nc.vector.tensor_copy(
                    g_im.rearrange("p (kl s xl) -> p s kl xl", kl=8, s=8, xl=8),
                    g_rim.rearrange("p (s kl xl) -> p s kl xl", s=8, kl=8, xl=8))

                tap("dbg_g2re", g_re); tap("dbg_g2im", g_im)
                t2_re, t2_im = tstage(fsb, tps, g_re, g_im, False)  # [x | kzl*64+ky]
                tap("dbg_t2re", t2_re); tap("dbg_t2im", t2_im)
                # P3 forward x, then filter fused into the PSUM->SBUF copy
                ps3_re = fps.tile([N, 512], FP, tag="psre")
                ps3_im = fps.tile([N, 512], FP, tag="psim")
                nc.tensor.matmul(ps3_re[:], lhsT=ArT, rhs=t2_re[:], start=True, stop=False)
                nc.tensor.matmul(ps3_re[:], lhsT=AiTn, rhs=t2_im[:], start=False, stop=True)
                nc.tensor.matmul(ps3_im[:], lhsT=AiT, rhs=t2_re[:], start=True, stop=False)
                nc.tensor.matmul(ps3_im[:], lhsT=ArT, rhs=t2_im[:], start=False, stop=True)
                f3_re = fsb.tile([N, 512], FR, tag="fre")
                f3_im = fsb.tile([N, 512], FR, tag="fim")
                nc.vector.tensor_tensor(f3_re[:], ps3_re[:], ham_sb[:], op=mybir.AluOpType.mult)
                nc.vector.tensor_tensor(f3_im[:], ps3_im[:], ham_sb[:], op=mybir.AluOpType.mult)

                tap("dbg_f3re", f3_re); tap("dbg_f3im", f3_im)
                g4_re, g4_im = cpass(fsb, fps, BrT, BiTn, BiT, f3_re, f3_im)  # [x | kzl*64+ky]
                t3_re, t3_im = tstage(fsb, tps, g4_re, g4_im, True)  # [ky | x*8+kzl]
                f5s = cpass(fsb, fps, BrT, BiTn, BiT, t3_re, t3_im, stacked=True)  # [y± | x*8+kzl]

                # A2A #2: -> [y | xl*64 + kz]   (chunk for dest d = cols [64d,64d+64))
                a2_in = dram.tile([N_CORES, 2, N, 8, 8], FR, tag="a2a2_in")
                a2_out = dram.tile([N_CORES, 2, N, 8, 8], FR, tag="a2a2_out")
                for dd in range(N_CORES):
                    dma_engines[dd % 4].dma_start(
                        a2_in[dd].rearrange("q p xl kl -> (q p) xl kl"),
                        f5s[:, dd * N:(dd + 1) * N].rearrange("p (xl kl) -> p xl kl", xl=8, kl=8))
                nc.gpsimd.collective_compute(
                    "AllToAll", mybir.AluOpType.bypass,
                    replica_groups=[list(range(N_CORES))],
                    ins=[a2_in.opt()], outs=[a2_out.opt()])
                # recv contiguously, then DVE-permute [p | s*64+xl*8+kl] -> [p | xl*64+s*8+kl]
                g5_raw = fsb.tile([128, 512], FR, tag="graw")
                for ss in range(N_CORES):
                    dma_engines[ss % 4].dma_start(
                        g5_raw[:, ss * N:(ss + 1) * N].rearrange("p (xl kl) -> p xl kl", xl=8, kl=8),
                        a2_out[ss].rearrange("q p xl kl -> (q p) xl kl"))
                g5_re = fsb.tile([N, 512], FR, tag="fre")
                g5_im = fsb.tile([N, 512], FR, tag="fim")
                nc.vector.tensor_copy(
                    g5_re.rearrange("p (xl s kl) -> p s xl kl", xl=8, s=8, kl=8),
                    g5_raw[0:N, :].rearrange("p (s xl kl) -> p s xl kl", s=8, xl=8, kl=8))
                nc.vector.tensor_copy(
                    g5_im.rearrange("p (xl s kl) -> p s xl kl", xl=8, s=8, kl=8),
                    g5_raw[N:128, :].rearrange("p (s xl kl) -> p s xl kl", s=8, xl=8, kl=8))

                tap("dbg_g5re", g5_re); tap("dbg_g5im", g5_im)
                t4_re, t4_im = tstage(fsb, tps, g5_re, g5_im, False)  # [kz | xl*64+y]
                # P6: inverse z, real part only
                ps6 = ps6p.tile([N, 512], FP, tag="ps6")
                nc.tensor.matmul(ps6[:], lhsT=BrT, rhs=t4_re[:], start=True, stop=False)
                nc.tensor.matmul(ps6[:], lhsT=BiTn, rhs=t4_im[:], start=False, stop=True)
                out_sb = fsb.tile([N, 512], FP, tag="osb")
                nc.vector.tensor_copy(out_sb[:], ps6[:])
                nc.sync.dma_start(out[:], out_sb[:])
            fsb_pool.__exit__(None, None, None)

    nc.compile()
    return nc


def _get_compiled():
    global _COMPILED
    if _COMPILED is None:
        _COMPILED = _build()
    return _COMPILED


def _host_inputs(X, aw, bw, real_grid_flat, hamming):
    X = np.asarray(X, np.float32)
    aw = np.asarray(aw, np.float32)
    bw = np.asarray(bw, np.float32)
    grid = np.asarray(real_grid_flat, np.float32)
    hamming = np.asarray(hamming, np.float32)

    arr = grid.reshape(N, N, N, 3)                       # [x, y, z, 3]
    xs = arr[:, 0, 0, 0]                                 # (64,)
    ys = arr[0, :, 0, 1]
    zs = arr[0, 0, :, 2]
    d2y = (ys[None, :] - X[:, 1:2]) ** 2                 # (128, 64)
    d2z = (zs[None, :] - X[:, 2:3]) ** 2                 # (128, 64)

    lnaw3 = (np.log(np.maximum(aw, 1e-38)) / 3.0).astype(np.float32)
    bwln = np.concatenate([bw, lnaw3], 1).astype(np.float32)    # (128, 12)

    F = np.fft.fft(np.eye(N), axis=0, norm='ortho')
    IF = np.fft.ifft(np.eye(N), axis=0, norm='ortho')
    perm = (np.arange(N) + N // 2) % N
    Am = F[:, perm]
    Bm = IF[perm, :]
    Ar, Ai = Am.real.astype(np.float32), Am.imag.astype(np.float32)
    Br, Bi = Bm.real.astype(np.float32), Bm.imag.astype(np.float32)
    Az2T = np.concatenate([Ar.T, Ai.T], 1)               # (64, 128)
    mats = np.concatenate(
        [Az2T, Ar.T, Ai.T, -Ai.T, Br.T, Bi.T, -Bi.T,
         np.eye(N, dtype=np.float32)], 1)                # (64, 576)

    Hfull = np.fft.ifftshift(hamming)                    # [kx, ky, kz]

    in_maps = []
    for c in range(N_CORES):
        d2x = (xs[None, 8 * c:8 * (c + 1)] - X[:, 0:1]) ** 2    # (128, 8)
        d2all = np.concatenate([d2x, d2y, d2z], 1).astype(np.float32)
        Hc = np.ascontiguousarray(
            np.transpose(Hfull[:, :, 8 * c:8 * (c + 1)], (0, 2, 1))).reshape(N, 512)
        in_maps.append({"d2all": d2all, "bwln": bwln,
                        "mats": mats, "ham": Hc})
    return in_maps


def kernel(X, aw, bw, real_grid_flat, hamming):
    global LAST_EXEC_NS, LAST_RESULTS
    in_maps = _host_inputs(X, aw, bw, real_grid_flat, hamming)
    nc = _get_compiled()

    trace = bool(os.environ.get("BASS_TRACE"))
    res = run_bass_kernel_spmd(nc, in_maps, core_ids=list(range(N_CORES)),
                               trace=trace)
    LAST_EXEC_NS = res.exec_time_ns
    global LAST_RESULTS
    LAST_RESULTS = res.results

    full = np.empty((N, N, N), np.float32)               # [z, x, y]
    for c in range(N_CORES):
        full[:, 8 * c:8 * (c + 1), :] = res.results[c]["out"].reshape(N, 8, N)
    o = np.transpose(full, (1, 2, 0))                    # [x, y, z]
    o = (o - o.mean()) / (o.std() + 1e-8)
    return o.astype(np.float32)



# revision 19
# speedup vs baseline: 2.4091x; 1.1094x over previous
"""Trainium2 Bass kernel for DensityCalculator.

density[g] = sum_a sum_k aw[a,k]*exp(bw[a,k]*|g-x_a|^2) over a 64^3 grid,
then 3D FFT -> hamming filter -> inverse FFT -> normalize.

Distribution: grid x-axis sharded over 8 cores (8 x-planes each) for the
density phase; FFT is a distributed pencil decomposition (z,y transforms
local to the x-slab, AllToAll to kz-slabs, x transform + filter + inverse x,
AllToAll back, inverse y,z local).

Device algebra (separable Gaussian splatting):
 - the Gaussian factors per axis: aw*exp(bw*|g-x|^2) =
   prod_axis exp(bw*d_axis^2 + ln(aw)/3), so only 136 distinct 1-D
   coordinate offsets per core (8 slab-x + 64 y + 64 z) need exps:
   6 scalar-engine Exp ops over [128 atoms, 136] with per-partition
   scale=bw[:,k], bias=ln(aw[:,k])/3
 - G_k[a, (x,y)] = Ex_k (x) Ey_k outer product via one DVE broadcast
   multiply per k (bf16)
 - rho[z, (x,y)] = sum_k sum_a Ez_k[a,z] * G_k[a,(x,y)]: 6 accumulating
   bf16 matmuls with Ez_k as lhsT (atom contraction on the PE)
 - FFT as matmuls with 64x64 DFT matrices (fftshift/ifftshift folded into
   column/row permutations host-side), transposes on TensorE.
"""

import os
import sys
import numpy as np

for _p in ("/opt/trn_rl_repo", "/root/.axon_site", "/root/.axon_site/_ro/trn_rl_repo",
           "/root/.axon_site/_ro/pypackages"):
    if _p not in sys.path and os.path.isdir(_p):
        sys.path.append(_p)

import concourse.bass as bass
import concourse.tile as tile
from concourse import bacc, mybir
from concourse.bass_utils import run_bass_kernel_spmd

FP = mybir.dt.float32
FR = mybir.dt.float32r
BF = mybir.dt.bfloat16
Exp = mybir.ActivationFunctionType.Exp

N_CORES = 8
N = 64              # grid size per axis
A = 128             # atoms
K = 6               # gaussian terms
XL = 8              # x-planes per core
GPC = N * XL * N    # grid points per core (32768)
D1 = XL + 2 * N     # 1-D separable offsets per core (136: x-slab, y, z)

LAST_EXEC_NS = None
LAST_RESULTS = None
_COMPILED = None
DEBUG_TAPS = bool(os.environ.get("KERNEL_DEBUG_TAPS"))


def _build():
    nc = bacc.Bacc("TRN2", target_bir_lowering=False, debug=False,
                   num_devices=N_CORES)

    d2all = nc.dram_tensor("d2all", [A, D1], FP, kind="ExternalInput").ap()
    bwln = nc.dram_tensor("bwln", [A, 2 * K], FP, kind="ExternalInput").ap()
    mats = nc.dram_tensor("mats", [N, 128 + 7 * N], FR, kind="ExternalInput").ap()
    ham = nc.dram_tensor("ham", [N, 512], FR, kind="ExternalInput").ap()
    out = nc.dram_tensor("out", [N, 512], FP, kind="ExternalOutput").ap()
    taps = {}
    if DEBUG_TAPS:
        for nm in ("dbg_rho", "dbg_f1re", "dbg_f1im", "dbg_t1re", "dbg_t1im",
                   "dbg_f2re", "dbg_f2im", "dbg_g2re", "dbg_g2im",
                   "dbg_t2re", "dbg_t2im", "dbg_f3re", "dbg_f3im",
                   "dbg_g5re", "dbg_g5im"):
            taps[nm] = nc.dram_tensor(nm, [N, 512], FR, kind="ExternalOutput").ap()

    def tap(nm, tile_):
        if DEBUG_TAPS:
            nc.sync.dma_start(taps[nm], tile_[:])

    with tile.TileContext(nc) as tc:
        with tc.tile_pool(name="const", bufs=1) as constp, \
             tc.tile_pool(name="dram", bufs=1, space="DRAM") as dram:
            d2_sb = constp.tile([A, D1], FP)
            nc.sync.dma_start(d2_sb[:], d2all[:])
            bwln_sb = constp.tile([A, 2 * K], FP)
            nc.sync.dma_start(bwln_sb[:], bwln[:])
            mats_sb = constp.tile([N, 128 + 7 * N], FR)
            nc.scalar.dma_start(mats_sb[:], mats[:])
            ham_sb = constp.tile([N, 512], FR)
            nc.scalar.dma_start(ham_sb[:], ham[:])

            # tiny warmup AllToAll so ncfw channel setup + cross-core skew
            # absorb as early as possible; payload is garbage DRAM (never
            # read), so no staging is needed and this issues immediately.
            wu_in = dram.tile([N_CORES, 8], FP, tag="wu_in")
            wu_out = dram.tile([N_CORES, 8], FP, tag="wu_out")
            nc.gpsimd.collective_compute(
                "AllToAll", mybir.AluOpType.bypass,
                replica_groups=[list(range(N_CORES))],
                ins=[wu_in.opt()], outs=[wu_out.opt()])

            # stationary views into mats
            Az2T = mats_sb[:, 0:128]
            c0 = 128
            ArT = mats_sb[:, c0:c0 + N]
            AiT = mats_sb[:, c0 + N:c0 + 2 * N]
            AiTn = mats_sb[:, c0 + 2 * N:c0 + 3 * N]
            BrT = mats_sb[:, c0 + 3 * N:c0 + 4 * N]
            BiT = mats_sb[:, c0 + 4 * N:c0 + 5 * N]
            BiTn = mats_sb[:, c0 + 5 * N:c0 + 6 * N]
            ident = mats_sb[:, c0 + 6 * N:c0 + 7 * N]

            # ---------------- Phase 1: separable density ----------------
            acc_pool = tc.tile_pool(name="p1acc", bufs=1, space="PSUM")
            accps = acc_pool.__enter__().tile([128, 512], FP, tag="acc")
            with tc.tile_pool(name="p1sb", bufs=1) as p1sb, \
                 tc.tile_pool(name="p1rho", bufs=1, space="PSUM") as p1rho:
                # E[a, k, :] = exp(bw_k*d2 + ln(aw_k)/3) over the 136 1-D
                # offsets; each of the three axis factors carries aw^(1/3)
                # so their product carries aw exactly once.
                E = p1sb.tile([A, K, D1], BF, tag="E")
                for k in range(K):
                    nc.scalar.activation(E[:, k, :], d2_sb[:], Exp,
                                         bias=bwln_sb[:, K + k:K + k + 1],
                                         scale=bwln_sb[:, k:k + 1])
                # G_k[a, x, y] = Ex_k (x) Ey_k  (DVE stride-0 broadcast mult)
                G = p1sb.tile([A, K, XL, N], BF, tag="G")
                for k in range(K):
                    nc.vector.tensor_tensor(
                        G[:, k],
                        E[:, k, 0:XL][:, :, None].broadcast_to([A, XL, N]),
                        E[:, k, XL:XL + N][:, None, :].broadcast_to([A, XL, N]),
                        op=mybir.AluOpType.mult)
                # rho[z, (x,y)] = sum_k Ez_k^T @ G_k  (atom contraction)
                rho_ps = p1rho.tile([N, 512], FP, tag="rho")
                for k in range(K):
                    nc.tensor.matmul(rho_ps[:],
                                     lhsT=E[:, k, XL + N:XL + 2 * N],
                                     rhs=G[:, k].rearrange("p x y -> p (x y)"),
                                     start=(k == 0), stop=(k == K - 1))
                rho_sb = p1sb.tile([N, 512], FR, tag="rho_sb")
                nc.vector.tensor_copy(rho_sb[:], rho_ps[:])
                tap("dbg_rho", rho_sb)
                # forward z-FFT: accps[(kz_re|kz_im), (x,y)] = Az2T^T @ rho
                nc.tensor.matmul(accps[:], lhsT=Az2T, rhs=rho_sb[:],
                                 start=True, stop=True)

            # ---------------- Phase 2: distributed FFT ----------------
            def cpass(fsb, fpsp, sre, sim_n, sim_, re_in, im_in, stacked=False):
                """complex pass: re_out = sre@re + sim_n@im ; im_out = sim_@re + sre@im
                (sim_n = negated imag matrix). Round-copies to f32r tiles.
                stacked=True returns one (128,512) tile [re | im] for A2A staging."""
                ps_re = fpsp.tile([N, 512], FP, tag="psre")
                ps_im = fpsp.tile([N, 512], FP, tag="psim")
                nc.tensor.matmul(ps_re[:], lhsT=sre, rhs=re_in[:], start=True, stop=False)
                nc.tensor.matmul(ps_re[:], lhsT=sim_n, rhs=im_in[:], start=False, stop=True)
                nc.tensor.matmul(ps_im[:], lhsT=sim_, rhs=re_in[:], start=True, stop=False)
                nc.tensor.matmul(ps_im[:], lhsT=sre, rhs=im_in[:], start=False, stop=True)
                if stacked:
                    o = fsb.tile([128, 512], FR, tag="fstk")
                    nc.vector.tensor_copy(o[0:N, :], ps_re[:])
                    nc.vector.tensor_copy(o[N:128, :], ps_im[:])
                    return o
                o_re = fsb.tile([N, 512], FR, tag="fre")
                o_im = fsb.tile([N, 512], FR, tag="fim")
                nc.vector.tensor_copy(o_re[:], ps_re[:])
                nc.vector.tensor_copy(o_im[:], ps_im[:])
                return o_re, o_im

            dma_engines = (nc.sync, nc.scalar, nc.gpsimd)

            def tstage(fsb, tps, re_in, im_in, strided_out):
                """transpose stage. Input [p | blk*64+q] (blk outer, q inner 64).
                strided_out=False: out[q | blk*64+p]  (contiguous 64-col writes)
                strided_out=True:  out[q | p*8+blk]   (stride-8 writes, 8 blocks)
                All 4 transposed blocks land in one PSUM tile so the
                PSUM->SBUF moves are 2 big strided copies instead of 8.
                """
                o_re = fsb.tile([N, 512], FR, tag="tre")
                o_im = fsb.tile([N, 512], FR, tag="tim")
                for (src, dst) in ((re_in, o_re), (im_in, o_im)):
                    pt = tps.tile([128, 4, N], FR, tag="pt")
                    for t in range(4):
                        nc.tensor.transpose(pt[:, t, :], src[:, t * 128:(t + 1) * 128],
                                            ident)
                    # block blk = 2t+h: dst cols from pt[64h:64h+64, t, :]
                    if strided_out:
                        dstv = dst.rearrange("w (i t h) -> w t i h", i=N, t=4, h=2)
                        for h in range(2):
                            nc.vector.tensor_copy(dstv[:, :, :, h],
                                                  pt[h * N:(h + 1) * N])
                    else:
                        dstv = dst.rearrange("w (t h i) -> w t h i", t=4, h=2, i=N)
                        for h in range(2):
                            nc.vector.tensor_copy(dstv[:, :, h],
                                                  pt[h * N:(h + 1) * N])
                return o_re, o_im

            fsb_pool = tc.tile_pool(name="fft", bufs=2)
            fsb = fsb_pool.__enter__()
            # forward z was computed in phase 1; round-copy out
            f_re = fsb.tile([N, 512], FR, tag="fre")
            f_im = fsb.tile([N, 512], FR, tag="fim")
            nc.vector.tensor_copy(f_re[:], accps[0:N, :])
            nc.vector.tensor_copy(f_im[:], accps[N:128, :])
            acc_pool.__exit__(None, None, None)
            with tc.tile_pool(name="fps", bufs=2, space="PSUM") as fps, \
                 tc.tile_pool(name="ps6p", bufs=1, space="PSUM") as ps6p, \
                 tc.tile_pool(name="tps", bufs=3, space="PSUM") as tps:
                tap("dbg_f1re", f_re); tap("dbg_f1im", f_im)
                t_re, t_im = tstage(fsb, tps, f_re, f_im, True)   # [y | kz*8+xl]
                tap("dbg_t1re", t_re); tap("dbg_t1im", t_im)
                f2s = cpass(fsb, fps, ArT, AiTn, AiT, t_re, t_im,
                            stacked=True)                 # [ky± | kz*8+xl]

                # A2A #1: -> [ky | kzl*64 + x]   (chunk for dest d = cols [64d,64d+64))
                a_in = dram.tile([N_CORES, 2, N, 8, 8], FR, tag="a2a_in")
                a_out = dram.tile([N_CORES, 2, N, 8, 8], FR, tag="a2a_out")
                for dd in range(N_CORES):
                    dma_engines[dd % 3].dma_start(
                        a_in[dd].rearrange("q p kl xl -> (q p) kl xl"),
                        f2s[:, dd * N:(dd + 1) * N].rearrange(
                            "p (kl xl) -> p kl xl", kl=8, xl=8))
                nc.gpsimd.collective_compute(
                    "AllToAll", mybir.AluOpType.bypass,
                    replica_groups=[list(range(N_CORES))],
                    ins=[a_in.opt()], outs=[a_out.opt()])
                # recv contiguously (cheap descriptors), then DVE-permute
                # [p | s*64+kl*8+xl] -> [p | kl*64+s*8+xl]
                g_raw = fsb.tile([128, 512], FR, tag="graw")
                for ss in range(N_CORES):
                    dma_engines[ss % 3].dma_start(
                        g_raw[:, ss * N:(ss + 1) * N].rearrange(
                            "p (kl xl) -> p kl xl", kl=8, xl=8),
                        a_out[ss].rearrange("q p kl xl -> (q p) kl xl"))
                g_re = fsb.tile([N, 512], FR, tag="fre")
                g_im = fsb.tile([N, 512], FR, tag="fim")
                nc.vector.tensor_copy(
                    g_re.rearrange("p (kl s xl) -> p s kl xl", kl=8, s=8, xl=8),
                    g_raw[0:N, :].rearrange("p (s kl xl) -> p s kl xl", s=8, kl=8, xl=8))
                nc.vector.tensor_copy(
                    g_im.rearrange("p (kl s xl) -> p s kl xl", kl=8, s=8, xl=8),
                    g_raw[N:128, :].rearrange("p (s kl xl) -> p s kl xl", s=8, kl=8, xl=8))

                tap("dbg_g2re", g_re); tap("dbg_g2im", g_im)
                t2_re, t2_im = tstage(fsb, tps, g_re, g_im, False)  # [x | kzl*64+ky]
                tap("dbg_t2re", t2_re); tap("dbg_t2im", t2_im)
                # P3 forward x, then filter fused into the PSUM->SBUF copy
                ps3_re = fps.tile([N, 512], FP, tag="psre")
                ps3_im = fps.tile([N, 512], FP, tag="psim")
                nc.tensor.matmul(ps3_re[:], lhsT=ArT, rhs=t2_re[:], start=True, stop=False)
                nc.tensor.matmul(ps3_re[:], lhsT=AiTn, rhs=t2_im[:], start=False, stop=True)
                nc.tensor.matmul(ps3_im[:], lhsT=AiT, rhs=t2_re[:], start=True, stop=False)
                nc.tensor.matmul(ps3_im[:], lhsT=ArT, rhs=t2_im[:], start=False, stop=True)
                f3_re = fsb.tile([N, 512], FR, tag="fre")
                f3_im = fsb.tile([N, 512], FR, tag="fim")
                nc.vector.tensor_tensor(f3_re[:], ps3_re[:], ham_sb[:], op=mybir.AluOpType.mult)
                nc.vector.tensor_tensor(f3_im[:], ps3_im[:], ham_sb[:], op=mybir.AluOpType.mult)

                tap("dbg_f3re", f3_re); tap("dbg_f3im", f3_im)
                g4_re, g4_im = cpass(fsb, fps, BrT, BiTn, BiT, f3_re, f3_im)  # [x | kzl*64+ky]
                t3_re, t3_im = tstage(fsb, tps, g4_re, g4_im, True)  # [ky | x*8+kzl]
                f5s = cpass(fsb, fps, BrT, BiTn, BiT, t3_re, t3_im, stacked=True)  # [y± | x*8+kzl]

                # A2A #2: -> [y | xl*64 + kz]   (chunk for dest d = cols [64d,64d+64))
                a2_in = dram.tile([N_CORES, 2, N, 8, 8], FR, tag="a2a2_in")
                a2_out = dram.tile([N_CORES, 2, N, 8, 8], FR, tag="a2a2_out")
                for dd in range(N_CORES):
                    dma_engines[dd % 3].dma_start(
                        a2_in[dd].rearrange("q p xl kl -> (q p) xl kl"),
                        f5s[:, dd * N:(dd + 1) * N].rearrange("p (xl kl) -> p xl kl", xl=8, kl=8))
                nc.gpsimd.collective_compute(
                    "AllToAll", mybir.AluOpType.bypass,
                    replica_groups=[list(range(N_CORES))],
                    ins=[a2_in.opt()], outs=[a2_out.opt()])
                # recv contiguously, then DVE-permute [p | s*64+xl*8+kl] -> [p | xl*64+s*8+kl]
                g5_raw = fsb.tile([128, 512], FR, tag="graw")
                for ss in range(N_CORES):
                    dma_engines[ss % 3].dma_start(
                        g5_raw[:, ss * N:(ss + 1) * N].rearrange("p (xl kl) -> p xl kl", xl=8, kl=8),
                        a2_out[ss].rearrange("q p xl kl -> (q p) xl kl"))
                g5_re = fsb.tile([N, 512], FR, tag="fre")
                g5_im = fsb.tile([N, 512], FR, tag="fim")
                nc.vector.tensor_copy(
                    g5_re.rearrange("p (xl s kl) -> p s xl kl", xl=8, s=8, kl=8),
                    g5_raw[0:N, :].rearrange("p (s xl kl) -> p s xl kl", s=8, xl=8, kl=8))
                nc.vector.tensor_copy(
                    g5_im.rearrange("p (xl s kl) -> p s xl kl", xl=8, s=8, kl=8),
                    g5_raw[N:128, :].rearrange("p (s xl kl) -> p s xl kl", s=8, xl=8, kl=8))

                tap("dbg_g5re", g5_re); tap("dbg_g5im", g5_im)
                t4_re, t4_im = tstage(fsb, tps, g5_re, g5_im, False)  # [kz | xl*64+y]
                # P6: inverse z, real part only
                ps6 = ps6p.tile([N, 512], FP, tag="ps6")
                nc.tensor.matmul(ps6[:], lhsT=BrT, rhs=t4_re[:], start=True, stop=False)
                nc.tensor.matmul(ps6[:], lhsT=BiTn, rhs=t4_im[:], start=False, stop=True)
                out_sb = fsb.tile([N, 512], FP, tag="osb")
                nc.vector.tensor_copy(out_sb[:], ps6[:])
                nc.sync.dma_start(out[:], out_sb[:])
            fsb_pool.__exit__(None, None, None)

    nc.compile()
    return nc


def _get_compiled():
    global _COMPILED
    if _COMPILED is None:
        _COMPILED = _build()
    return _COMPILED


def _host_inputs(X, aw, bw, real_grid_flat, hamming):
    X = np.asarray(X, np.float32)
    aw = np.asarray(aw, np.float32)
    bw = np.asarray(bw, np.float32)
    grid = np.asarray(real_grid_flat, np.float32)
    hamming = np.asarray(hamming, np.float32)

    arr = grid.reshape(N, N, N, 3)                       # [x, y, z, 3]
    xs = arr[:, 0, 0, 0]                                 # (64,)
    ys = arr[0, :, 0, 1]
    zs = arr[0, 0, :, 2]
    d2y = (ys[None, :] - X[:, 1:2]) ** 2                 # (128, 64)
    d2z = (zs[None, :] - X[:, 2:3]) ** 2                 # (128, 64)

    lnaw3 = (np.log(np.maximum(aw, 1e-38)) / 3.0).astype(np.float32)
    bwln = np.concatenate([bw, lnaw3], 1).astype(np.float32)    # (128, 12)

    F = np.fft.fft(np.eye(N), axis=0, norm='ortho')
    IF = np.fft.ifft(np.eye(N), axis=0, norm='ortho')
    perm = (np.arange(N) + N // 2) % N
    Am = F[:, perm]
    Bm = IF[perm, :]
    Ar, Ai = Am.real.astype(np.float32), Am.imag.astype(np.float32)
    Br, Bi = Bm.real.astype(np.float32), Bm.imag.astype(np.float32)
    Az2T = np.concatenate([Ar.T, Ai.T], 1)               # (64, 128)
    mats = np.concatenate(
        [Az2T, Ar.T, Ai.T, -Ai.T, Br.T, Bi.T, -Bi.T,
         np.eye(N, dtype=np.float32)], 1)                # (64, 576)

    Hfull = np.fft.ifftshift(hamming)                    # [kx, ky, kz]

    in_maps = []
    for c in range(N_CORES):
        d2x = (xs[None, 8 * c:8 * (c + 1)] - X[:, 0:1]) ** 2    # (128, 8)
        d2all = np.concatenate([d2x, d2y, d2z], 1).astype(np.float32)
        Hc = np.ascontiguousarray(
            np.transpose(Hfull[:, :, 8 * c:8 * (c + 1)], (0, 2, 1))).reshape(N, 512)
        in_maps.append({"d2all": d2all, "bwln": bwln,
                        "mats": mats, "ham": Hc})
    return in_maps


def kernel(X, aw, bw, real_grid_flat, hamming):
    global LAST_EXEC_NS, LAST_RESULTS
    in_maps = _host_inputs(X, aw, bw, real_grid_flat, hamming)
    nc = _get_compiled()

    trace = bool(os.environ.get("BASS_TRACE"))
    res = run_bass_kernel_spmd(nc, in_maps, core_ids=list(range(N_CORES)),
                               trace=trace)
    LAST_EXEC_NS = res.exec_time_ns
    global LAST_RESULTS
    LAST_RESULTS = res.results

    full = np.empty((N, N, N), np.float32)               # [z, x, y]
    for c in range(N_CORES):
        full[:, 8 * c:8 * (c + 1), :] = res.results[c]["out"].reshape(N, 8, N)
    o = np.transpose(full, (1, 2, 0))                    # [x, y, z]
    o = (o - o.mean()) / (o.std() + 1e-8)
    return o.astype(np.float32)



# revision 20
# speedup vs baseline: 2.4888x; 1.0331x over previous
"""Trainium2 Bass kernel for DensityCalculator.

density[g] = sum_a sum_k aw[a,k]*exp(bw[a,k]*|g-x_a|^2) over a 64^3 grid,
then 3D FFT -> hamming filter -> inverse FFT -> normalize.

Distribution: grid x-axis sharded over 8 cores (8 x-planes each) for the
density phase; FFT is a distributed pencil decomposition (z,y transforms
local to the x-slab, AllToAll to kz-slabs, x transform + filter + inverse x,
AllToAll back, inverse y,z local).

Device algebra (separable Gaussian splatting):
 - the Gaussian factors per axis: aw*exp(bw*|g-x|^2) =
   prod_axis exp(bw*d_axis^2 + ln(aw)/3), so only 136 distinct 1-D
   coordinate offsets per core (8 slab-x + 64 y + 64 z) need exps:
   6 scalar-engine Exp ops over [128 atoms, 136] with per-partition
   scale=bw[:,k], bias=ln(aw[:,k])/3
 - G_k[a, (x,y)] = Ex_k (x) Ey_k outer product via one DVE broadcast
   multiply per k (bf16)
 - rho[z, (x,y)] = sum_k sum_a Ez_k[a,z] * G_k[a,(x,y)]: 6 accumulating
   bf16 matmuls with Ez_k as lhsT (atom contraction on the PE)
 - FFT as matmuls with 64x64 DFT matrices (fftshift/ifftshift folded into
   column/row permutations host-side), transposes on TensorE.
"""

import os
import sys
import numpy as np

for _p in ("/opt/trn_rl_repo", "/root/.axon_site", "/root/.axon_site/_ro/trn_rl_repo",
           "/root/.axon_site/_ro/pypackages"):
    if _p not in sys.path and os.path.isdir(_p):
        sys.path.append(_p)

import concourse.bass as bass
import concourse.tile as tile
from concourse import bacc, mybir
from concourse.bass_utils import run_bass_kernel_spmd

FP = mybir.dt.float32
FR = mybir.dt.float32r
BF = mybir.dt.bfloat16
Exp = mybir.ActivationFunctionType.Exp

N_CORES = 8
N = 64              # grid size per axis
A = 128             # atoms
K = 6               # gaussian terms
XL = 8              # x-planes per core
GPC = N * XL * N    # grid points per core (32768)
D1 = XL + 2 * N     # 1-D separable offsets per core (136: x-slab, y, z)

LAST_EXEC_NS = None
LAST_RESULTS = None
_COMPILED = None
DEBUG_TAPS = bool(os.environ.get("KERNEL_DEBUG_TAPS"))


def _build():
    nc = bacc.Bacc("TRN2", target_bir_lowering=False, debug=False,
                   num_devices=N_CORES)

    d2all = nc.dram_tensor("d2all", [A, D1], FP, kind="ExternalInput").ap()
    bwln = nc.dram_tensor("bwln", [A, 2 * K], FP, kind="ExternalInput").ap()
    mats = nc.dram_tensor("mats", [N, 128 + 7 * N], FR, kind="ExternalInput").ap()
    ham = nc.dram_tensor("ham", [N, 512], FR, kind="ExternalInput").ap()
    out = nc.dram_tensor("out", [N, 512], FP, kind="ExternalOutput").ap()
    taps = {}
    if DEBUG_TAPS:
        for nm in ("dbg_rho", "dbg_f1re", "dbg_f1im", "dbg_t1re", "dbg_t1im",
                   "dbg_f2re", "dbg_f2im", "dbg_g2re", "dbg_g2im",
                   "dbg_t2re", "dbg_t2im", "dbg_f3re", "dbg_f3im",
                   "dbg_g5re", "dbg_g5im"):
            taps[nm] = nc.dram_tensor(nm, [N, 512], FR, kind="ExternalOutput").ap()

    def tap(nm, tile_):
        if DEBUG_TAPS:
            nc.sync.dma_start(taps[nm], tile_[:])

    with tile.TileContext(nc) as tc:
        with tc.tile_pool(name="const", bufs=1) as constp, \
             tc.tile_pool(name="dram", bufs=1, space="DRAM") as dram:
            d2_sb = constp.tile([A, D1], FP)
            nc.sync.dma_start(d2_sb[:], d2all[:])
            bwln_sb = constp.tile([A, 2 * K], FP)
            nc.sync.dma_start(bwln_sb[:], bwln[:])
            mats_sb = constp.tile([N, 128 + 7 * N], FR)
            nc.scalar.dma_start(mats_sb[:], mats[:])
            ham_sb = constp.tile([N, 512], FR)
            nc.scalar.dma_start(ham_sb[:], ham[:])

            # (no warmup collective: the first real A2A pays the one-time
            # ncfw mesh setup directly; a warmup would only serialize in
            # front of it on the CC engine and delay it further.)

            # stationary views into mats
            Az2T = mats_sb[:, 0:128]
            c0 = 128
            ArT = mats_sb[:, c0:c0 + N]
            AiT = mats_sb[:, c0 + N:c0 + 2 * N]
            AiTn = mats_sb[:, c0 + 2 * N:c0 + 3 * N]
            BrT = mats_sb[:, c0 + 3 * N:c0 + 4 * N]
            BiT = mats_sb[:, c0 + 4 * N:c0 + 5 * N]
            BiTn = mats_sb[:, c0 + 5 * N:c0 + 6 * N]
            ident = mats_sb[:, c0 + 6 * N:c0 + 7 * N]

            # ---------------- Phase 1: separable density ----------------
            acc_pool = tc.tile_pool(name="p1acc", bufs=1, space="PSUM")
            accps = acc_pool.__enter__().tile([128, 512], FP, tag="acc")
            with tc.tile_pool(name="p1sb", bufs=1) as p1sb, \
                 tc.tile_pool(name="p1rho", bufs=1, space="PSUM") as p1rho:
                # E[a, k, :] = exp(bw_k*d2 + ln(aw_k)/3) over the 136 1-D
                # offsets; each of the three axis factors carries aw^(1/3)
                # so their product carries aw exactly once.
                E = p1sb.tile([A, K, D1], BF, tag="E")
                for k in range(K):
                    nc.scalar.activation(E[:, k, :], d2_sb[:], Exp,
                                         bias=bwln_sb[:, K + k:K + k + 1],
                                         scale=bwln_sb[:, k:k + 1])
                # G_k[a, x, y] = Ex_k (x) Ey_k  (DVE stride-0 broadcast mult)
                G = p1sb.tile([A, K, XL, N], BF, tag="G")
                for k in range(K):
                    nc.vector.tensor_tensor(
                        G[:, k],
                        E[:, k, 0:XL][:, :, None].broadcast_to([A, XL, N]),
                        E[:, k, XL:XL + N][:, None, :].broadcast_to([A, XL, N]),
                        op=mybir.AluOpType.mult)
                # rho[z, (x,y)] = sum_k Ez_k^T @ G_k  (atom contraction)
                rho_ps = p1rho.tile([N, 512], FP, tag="rho")
                for k in range(K):
                    nc.tensor.matmul(rho_ps[:],
                                     lhsT=E[:, k, XL + N:XL + 2 * N],
                                     rhs=G[:, k].rearrange("p x y -> p (x y)"),
                                     start=(k == 0), stop=(k == K - 1))
                rho_sb = p1sb.tile([N, 512], FR, tag="rho_sb")
                nc.vector.tensor_copy(rho_sb[:], rho_ps[:])
                tap("dbg_rho", rho_sb)
                # forward z-FFT: accps[(kz_re|kz_im), (x,y)] = Az2T^T @ rho
                nc.tensor.matmul(accps[:], lhsT=Az2T, rhs=rho_sb[:],
                                 start=True, stop=True)

            # ---------------- Phase 2: distributed FFT ----------------
            def cpass(fsb, fpsp, sre, sim_n, sim_, re_in, im_in, stacked=False):
                """complex pass: re_out = sre@re + sim_n@im ; im_out = sim_@re + sre@im
                (sim_n = negated imag matrix). Round-copies to f32r tiles.
                stacked=True returns one (128,512) tile [re | im] for A2A staging."""
                ps_re = fpsp.tile([N, 512], FP, tag="psre")
                ps_im = fpsp.tile([N, 512], FP, tag="psim")
                nc.tensor.matmul(ps_re[:], lhsT=sre, rhs=re_in[:], start=True, stop=False)
                nc.tensor.matmul(ps_re[:], lhsT=sim_n, rhs=im_in[:], start=False, stop=True)
                nc.tensor.matmul(ps_im[:], lhsT=sim_, rhs=re_in[:], start=True, stop=False)
                nc.tensor.matmul(ps_im[:], lhsT=sre, rhs=im_in[:], start=False, stop=True)
                if stacked:
                    o = fsb.tile([128, 512], FR, tag="fstk")
                    nc.vector.tensor_copy(o[0:N, :], ps_re[:])
                    nc.vector.tensor_copy(o[N:128, :], ps_im[:])
                    return o
                o_re = fsb.tile([N, 512], FR, tag="fre")
                o_im = fsb.tile([N, 512], FR, tag="fim")
                nc.vector.tensor_copy(o_re[:], ps_re[:])
                nc.vector.tensor_copy(o_im[:], ps_im[:])
                return o_re, o_im

            dma_engines = (nc.sync, nc.scalar, nc.gpsimd)

            def tstage(fsb, tps, re_in, im_in, strided_out):
                """transpose stage. Input [p | blk*64+q] (blk outer, q inner 64).
                strided_out=False: out[q | blk*64+p]  (contiguous 64-col writes)
                strided_out=True:  out[q | p*8+blk]   (stride-8 writes, 8 blocks)
                All 4 transposed blocks land in one PSUM tile so the
                PSUM->SBUF moves are 2 big strided copies instead of 8.
                """
                o_re = fsb.tile([N, 512], FR, tag="tre")
                o_im = fsb.tile([N, 512], FR, tag="tim")
                for (src, dst) in ((re_in, o_re), (im_in, o_im)):
                    pt = tps.tile([128, 4, N], FR, tag="pt")
                    for t in range(4):
                        nc.tensor.transpose(pt[:, t, :], src[:, t * 128:(t + 1) * 128],
                                            ident)
                    # block blk = 2t+h: dst cols from pt[64h:64h+64, t, :]
                    if strided_out:
                        dstv = dst.rearrange("w (i t h) -> w t i h", i=N, t=4, h=2)
                        for h in range(2):
                            nc.vector.tensor_copy(dstv[:, :, :, h],
                                                  pt[h * N:(h + 1) * N])
                    else:
                        dstv = dst.rearrange("w (t h i) -> w t h i", t=4, h=2, i=N)
                        for h in range(2):
                            nc.vector.tensor_copy(dstv[:, :, h],
                                                  pt[h * N:(h + 1) * N])
                return o_re, o_im

            fsb_pool = tc.tile_pool(name="fft", bufs=2)
            fsb = fsb_pool.__enter__()
            # forward z was computed in phase 1; round-copy out
            f_re = fsb.tile([N, 512], FR, tag="fre")
            f_im = fsb.tile([N, 512], FR, tag="fim")
            nc.vector.tensor_copy(f_re[:], accps[0:N, :])
            nc.vector.tensor_copy(f_im[:], accps[N:128, :])
            acc_pool.__exit__(None, None, None)
            with tc.tile_pool(name="fps", bufs=2, space="PSUM") as fps, \
                 tc.tile_pool(name="ps6p", bufs=1, space="PSUM") as ps6p, \
                 tc.tile_pool(name="tps", bufs=3, space="PSUM") as tps:
                tap("dbg_f1re", f_re); tap("dbg_f1im", f_im)
                t_re, t_im = tstage(fsb, tps, f_re, f_im, True)   # [y | kz*8+xl]
                tap("dbg_t1re", t_re); tap("dbg_t1im", t_im)
                f2s = cpass(fsb, fps, ArT, AiTn, AiT, t_re, t_im,
                            stacked=True)                 # [ky± | kz*8+xl]

                # A2A #1: -> [ky | kzl*64 + x]   (chunk for dest d = cols [64d,64d+64))
                a_in = dram.tile([N_CORES, 2, N, 8, 8], FR, tag="a2a_in")
                a_out = dram.tile([N_CORES, 2, N, 8, 8], FR, tag="a2a_out")
                for dd in range(N_CORES):
                    dma_engines[dd % 3].dma_start(
                        a_in[dd].rearrange("q p kl xl -> (q p) kl xl"),
                        f2s[:, dd * N:(dd + 1) * N].rearrange(
                            "p (kl xl) -> p kl xl", kl=8, xl=8))
                nc.gpsimd.collective_compute(
                    "AllToAll", mybir.AluOpType.bypass,
                    replica_groups=[list(range(N_CORES))],
                    ins=[a_in.opt()], outs=[a_out.opt()])
                # recv contiguously (cheap descriptors), then DVE-permute
                # [p | s*64+kl*8+xl] -> [p | kl*64+s*8+xl]
                g_raw = fsb.tile([128, 512], FR, tag="graw")
                for ss in range(N_CORES):
                    dma_engines[ss % 3].dma_start(
                        g_raw[:, ss * N:(ss + 1) * N].rearrange(
                            "p (kl xl) -> p kl xl", kl=8, xl=8),
                        a_out[ss].rearrange("q p kl xl -> (q p) kl xl"))
                g_re = fsb.tile([N, 512], FR, tag="fre")
                g_im = fsb.tile([N, 512], FR, tag="fim")
                nc.vector.tensor_copy(
                    g_re.rearrange("p (kl s xl) -> p s kl xl", kl=8, s=8, xl=8),
                    g_raw[0:N, :].rearrange("p (s kl xl) -> p s kl xl", s=8, kl=8, xl=8))
                nc.vector.tensor_copy(
                    g_im.rearrange("p (kl s xl) -> p s kl xl", kl=8, s=8, xl=8),
                    g_raw[N:128, :].rearrange("p (s kl xl) -> p s kl xl", s=8, kl=8, xl=8))

                tap("dbg_g2re", g_re); tap("dbg_g2im", g_im)
                t2_re, t2_im = tstage(fsb, tps, g_re, g_im, False)  # [x | kzl*64+ky]
                tap("dbg_t2re", t2_re); tap("dbg_t2im", t2_im)
                # P3 forward x, then filter fused into the PSUM->SBUF copy
                ps3_re = fps.tile([N, 512], FP, tag="psre")
                ps3_im = fps.tile([N, 512], FP, tag="psim")
                nc.tensor.matmul(ps3_re[:], lhsT=ArT, rhs=t2_re[:], start=True, stop=False)
                nc.tensor.matmul(ps3_re[:], lhsT=AiTn, rhs=t2_im[:], start=False, stop=True)
                nc.tensor.matmul(ps3_im[:], lhsT=AiT, rhs=t2_re[:], start=True, stop=False)
                nc.tensor.matmul(ps3_im[:], lhsT=ArT, rhs=t2_im[:], start=False, stop=True)
                f3_re = fsb.tile([N, 512], FR, tag="fre")
                f3_im = fsb.tile([N, 512], FR, tag="fim")
                nc.vector.tensor_tensor(f3_re[:], ps3_re[:], ham_sb[:], op=mybir.AluOpType.mult)
                nc.vector.tensor_tensor(f3_im[:], ps3_im[:], ham_sb[:], op=mybir.AluOpType.mult)

                tap("dbg_f3re", f3_re); tap("dbg_f3im", f3_im)
                g4_re, g4_im = cpass(fsb, fps, BrT, BiTn, BiT, f3_re, f3_im)  # [x | kzl*64+ky]
                t3_re, t3_im = tstage(fsb, tps, g4_re, g4_im, True)  # [ky | x*8+kzl]
                f5s = cpass(fsb, fps, BrT, BiTn, BiT, t3_re, t3_im, stacked=True)  # [y± | x*8+kzl]

                # A2A #2: -> [y | xl*64 + kz]   (chunk for dest d = cols [64d,64d+64))
                a2_in = dram.tile([N_CORES, 2, N, 8, 8], FR, tag="a2a2_in")
                a2_out = dram.tile([N_CORES, 2, N, 8, 8], FR, tag="a2a2_out")
                for dd in range(N_CORES):
                    dma_engines[dd % 3].dma_start(
                        a2_in[dd].rearrange("q p xl kl -> (q p) xl kl"),
                        f5s[:, dd * N:(dd + 1) * N].rearrange("p (xl kl) -> p xl kl", xl=8, kl=8))
                nc.gpsimd.collective_compute(
                    "AllToAll", mybir.AluOpType.bypass,
                    replica_groups=[list(range(N_CORES))],
                    ins=[a2_in.opt()], outs=[a2_out.opt()])
                # recv contiguously, then DVE-permute [p | s*64+xl*8+kl] -> [p | xl*64+s*8+kl]
                g5_raw = fsb.tile([128, 512], FR, tag="graw")
                for ss in range(N_CORES):
                    dma_engines[ss % 3].dma_start(
                        g5_raw[:, ss * N:(ss + 1) * N].rearrange("p (xl kl) -> p xl kl", xl=8, kl=8),
                        a2_out[ss].rearrange("q p xl kl -> (q p) xl kl"))
                g5_re = fsb.tile([N, 512], FR, tag="fre")
                g5_im = fsb.tile([N, 512], FR, tag="fim")
                nc.vector.tensor_copy(
                    g5_re.rearrange("p (xl s kl) -> p s xl kl", xl=8, s=8, kl=8),
                    g5_raw[0:N, :].rearrange("p (s xl kl) -> p s xl kl", s=8, xl=8, kl=8))
                nc.vector.tensor_copy(
                    g5_im.rearrange("p (xl s kl) -> p s xl kl", xl=8, s=8, kl=8),
                    g5_raw[N:128, :].rearrange("p (s xl kl) -> p s xl kl", s=8, xl=8, kl=8))

                tap("dbg_g5re", g5_re); tap("dbg_g5im", g5_im)
                t4_re, t4_im = tstage(fsb, tps, g5_re, g5_im, False)  # [kz | xl*64+y]
                # P6: inverse z, real part only
                ps6 = ps6p.tile([N, 512], FP, tag="ps6")
                nc.tensor.matmul(ps6[:], lhsT=BrT, rhs=t4_re[:], start=True, stop=False)
                nc.tensor.matmul(ps6[:], lhsT=BiTn, rhs=t4_im[:], start=False, stop=True)
                out_sb = fsb.tile([N, 512], FP, tag="osb")
                nc.vector.tensor_copy(out_sb[:], ps6[:])
                nc.sync.dma_start(out[:], out_sb[:])
            fsb_pool.__exit__(None, None, None)

    nc.compile()
    return nc


def _get_compiled():
    global _COMPILED
    if _COMPILED is None:
        _COMPILED = _build()
    return _COMPILED


def _host_inputs(X, aw, bw, real_grid_flat, hamming):
    X = np.asarray(X, np.float32)
    aw = np.asarray(aw, np.float32)
    bw = np.asarray(bw, np.float32)
    grid = np.asarray(real_grid_flat, np.float32)
    hamming = np.asarray(hamming, np.float32)

    arr = grid.reshape(N, N, N, 3)                       # [x, y, z, 3]
    xs = arr[:, 0, 0, 0]                                 # (64,)
    ys = arr[0, :, 0, 1]
    zs = arr[0, 0, :, 2]
    d2y = (ys[None, :] - X[:, 1:2]) ** 2                 # (128, 64)
    d2z = (zs[None, :] - X[:, 2:3]) ** 2                 # (128, 64)

    lnaw3 = (np.log(np.maximum(aw, 1e-38)) / 3.0).astype(np.float32)
    bwln = np.concatenate([bw, lnaw3], 1).astype(np.float32)    # (128, 12)

    F = np.fft.fft(np.eye(N), axis=0, norm='ortho')
    IF = np.fft.ifft(np.eye(N), axis=0, norm='ortho')
    perm = (np.arange(N) + N // 2) % N
    Am = F[:, perm]
    Bm = IF[perm, :]
    Ar, Ai = Am.real.astype(np.float32), Am.imag.astype(np.float32)
    Br, Bi = Bm.real.astype(np.float32), Bm.imag.astype(np.float32)
    Az2T = np.concatenate([Ar.T, Ai.T], 1)               # (64, 128)
    mats = np.concatenate(
        [Az2T, Ar.T, Ai.T, -Ai.T, Br.T, Bi.T, -Bi.T,
         np.eye(N, dtype=np.float32)], 1)                # (64, 576)

    Hfull = np.fft.ifftshift(hamming)                    # [kx, ky, kz]

    in_maps = []
    for c in range(N_CORES):
        d2x = (xs[None, 8 * c:8 * (c + 1)] - X[:, 0:1]) ** 2    # (128, 8)
        d2all = np.concatenate([d2x, d2y, d2z], 1).astype(np.float32)
        Hc = np.ascontiguousarray(
            np.transpose(Hfull[:, :, 8 * c:8 * (c + 1)], (0, 2, 1))).reshape(N, 512)
        in_maps.append({"d2all": d2all, "bwln": bwln,
                        "mats": mats, "ham": Hc})
    return in_maps


def kernel(X, aw, bw, real_grid_flat, hamming):
    global LAST_EXEC_NS, LAST_RESULTS
    in_maps = _host_inputs(X, aw, bw, real_grid_flat, hamming)
    nc = _get_compiled()

    trace = bool(os.environ.get("BASS_TRACE"))
    res = run_bass_kernel_spmd(nc, in_maps, core_ids=list(range(N_CORES)),
                               trace=trace)
    LAST_EXEC_NS = res.exec_time_ns
    global LAST_RESULTS
    LAST_RESULTS = res.results

    full = np.empty((N, N, N), np.float32)               # [z, x, y]
    for c in range(N_CORES):
        full[:, 8 * c:8 * (c + 1), :] = res.results[c]["out"].reshape(N, 8, N)
    o = np.transpose(full, (1, 2, 0))                    # [x, y, z]
    o = (o - o.mean()) / (o.std() + 1e-8)
    return o.astype(np.float32)



# revision 27
# speedup vs baseline: 2.5487x; 1.0241x over previous
"""Trainium2 Bass kernel for DensityCalculator.

density[g] = sum_a sum_k aw[a,k]*exp(bw[a,k]*|g-x_a|^2) over a 64^3 grid,
then 3D FFT -> hamming filter -> inverse FFT -> normalize.

Distribution: grid x-axis sharded over 8 cores (8 x-planes each) for the
density phase; FFT is a distributed pencil decomposition (z,y transforms
local to the x-slab, AllToAll to kz-slabs, x transform + filter + inverse x,
AllToAll back, inverse y,z local).

Device algebra (separable Gaussian splatting):
 - the Gaussian factors per axis: aw*exp(bw*|g-x|^2) =
   prod_axis exp(bw*d_axis^2 + ln(aw)/3), so only 136 distinct 1-D
   coordinate offsets per core (8 slab-x + 64 y + 64 z) need exps:
   6 scalar-engine Exp ops over [128 atoms, 136] with per-partition
   scale=bw[:,k], bias=ln(aw[:,k])/3
 - G_k[a, (x,y)] = Ex_k (x) Ey_k outer product via one DVE broadcast
   multiply per k (bf16)
 - rho[z, (x,y)] = sum_k sum_a Ez_k[a,z] * G_k[a,(x,y)]: 6 accumulating
   bf16 matmuls with Ez_k as lhsT (atom contraction on the PE)
 - FFT as matmuls with 64x64 DFT matrices (fftshift/ifftshift folded into
   column/row permutations host-side), transposes on TensorE.
"""

import os
import sys
import numpy as np

for _p in ("/opt/trn_rl_repo", "/root/.axon_site", "/root/.axon_site/_ro/trn_rl_repo",
           "/root/.axon_site/_ro/pypackages"):
    if _p not in sys.path and os.path.isdir(_p):
        sys.path.append(_p)

import concourse.bass as bass
import concourse.tile as tile
from concourse import bacc, mybir
from concourse.bass_utils import run_bass_kernel_spmd

FP = mybir.dt.float32
FR = mybir.dt.float32r
BF = mybir.dt.bfloat16
Exp = mybir.ActivationFunctionType.Exp

N_CORES = 8
N = 64              # grid size per axis
A = 128             # atoms
K = 6               # gaussian terms
XL = 8              # x-planes per core
GPC = N * XL * N    # grid points per core (32768)
D1 = XL + 2 * N     # 1-D separable offsets per core (136: x-slab, y, z)

LAST_EXEC_NS = None
LAST_RESULTS = None
_COMPILED = None
DEBUG_TAPS = bool(os.environ.get("KERNEL_DEBUG_TAPS"))


def _build():
    nc = bacc.Bacc("TRN2", target_bir_lowering=False, debug=False,
                   num_devices=N_CORES)

    d2all = nc.dram_tensor("d2all", [A, D1], FP, kind="ExternalInput").ap()
    bwln = nc.dram_tensor("bwln", [A, 2 * K], FP, kind="ExternalInput").ap()
    mats = nc.dram_tensor("mats", [N, 128 + 7 * N], FR, kind="ExternalInput").ap()
    ham = nc.dram_tensor("ham", [N, 512], FR, kind="ExternalInput").ap()
    out = nc.dram_tensor("out", [N, 512], FP, kind="ExternalOutput").ap()
    taps = {}
    if DEBUG_TAPS:
        for nm in ("dbg_rho", "dbg_f1re", "dbg_f1im", "dbg_t1re", "dbg_t1im",
                   "dbg_f2re", "dbg_f2im", "dbg_g2re", "dbg_g2im",
                   "dbg_t2re", "dbg_t2im", "dbg_f3re", "dbg_f3im",
                   "dbg_g5re", "dbg_g5im"):
            taps[nm] = nc.dram_tensor(nm, [N, 512], FR, kind="ExternalOutput").ap()

    def tap(nm, tile_):
        if DEBUG_TAPS:
            nc.sync.dma_start(taps[nm], tile_[:])

    with tile.TileContext(nc) as tc:
        with tc.tile_pool(name="const", bufs=1) as constp, \
             tc.tile_pool(name="dram", bufs=1, space="DRAM") as dram:
            d2_sb = constp.tile([A, D1], FP)
            nc.sync.dma_start(d2_sb[:], d2all[:])
            bwln_sb = constp.tile([A, 2 * K], FP)
            nc.sync.dma_start(bwln_sb[:], bwln[:])
            mats_sb = constp.tile([N, 128 + 7 * N], FR)
            nc.scalar.dma_start(mats_sb[:], mats[:])
            ham_sb = constp.tile([N, 512], FR)
            nc.scalar.dma_start(ham_sb[:], ham[:])

            # (no warmup collective: the first real A2A pays the one-time
            # ncfw mesh setup directly; a warmup would only serialize in
            # front of it on the CC engine and delay it further.)

            # stationary views into mats
            Az2T = mats_sb[:, 0:128]
            c0 = 128
            ArT = mats_sb[:, c0:c0 + N]
            AiT = mats_sb[:, c0 + N:c0 + 2 * N]
            AiTn = mats_sb[:, c0 + 2 * N:c0 + 3 * N]
            BrT = mats_sb[:, c0 + 3 * N:c0 + 4 * N]
            BiT = mats_sb[:, c0 + 4 * N:c0 + 5 * N]
            BiTn = mats_sb[:, c0 + 5 * N:c0 + 6 * N]
            ident = mats_sb[:, c0 + 6 * N:c0 + 7 * N]

            # ---------------- Phase 1: separable density ----------------
            acc_pool = tc.tile_pool(name="p1acc", bufs=1, space="PSUM")
            accps = acc_pool.__enter__().tile([128, 512], FP, tag="acc")
            with tc.tile_pool(name="p1sb", bufs=1) as p1sb, \
                 tc.tile_pool(name="p1rho", bufs=1, space="PSUM") as p1rho:
                # E[a, k, :] = exp(bw_k*d2 + ln(aw_k)/3) over the 136 1-D
                # offsets; each of the three axis factors carries aw^(1/3)
                # so their product carries aw exactly once.
                E = p1sb.tile([A, K, D1], BF, tag="E")
                for k in range(K):
                    nc.scalar.activation(E[:, k, :], d2_sb[:], Exp,
                                         bias=bwln_sb[:, K + k:K + k + 1],
                                         scale=bwln_sb[:, k:k + 1])
                # G_k[a, x, y] = Ex_k (x) Ey_k  (DVE stride-0 broadcast mult)
                G = p1sb.tile([A, K, XL, N], BF, tag="G")
                for k in range(K):
                    nc.vector.tensor_tensor(
                        G[:, k],
                        E[:, k, 0:XL][:, :, None].broadcast_to([A, XL, N]),
                        E[:, k, XL:XL + N][:, None, :].broadcast_to([A, XL, N]),
                        op=mybir.AluOpType.mult)
                # rho[z, (x,y)] = sum_k Ez_k^T @ G_k  (atom contraction)
                rho_ps = p1rho.tile([N, 512], FP, tag="rho")
                for k in range(K):
                    nc.tensor.matmul(rho_ps[:],
                                     lhsT=E[:, k, XL + N:XL + 2 * N],
                                     rhs=G[:, k].rearrange("p x y -> p (x y)"),
                                     start=(k == 0), stop=(k == K - 1))
                rho_sb = p1sb.tile([N, 512], FR, tag="rho_sb")
                nc.vector.tensor_copy(rho_sb[:], rho_ps[:])
                tap("dbg_rho", rho_sb)
                # forward z-FFT: accps[(kz_re|kz_im), (x,y)] = Az2T^T @ rho
                nc.tensor.matmul(accps[:], lhsT=Az2T, rhs=rho_sb[:],
                                 start=True, stop=True)

            # ---------------- Phase 2: distributed FFT ----------------
            Copy = mybir.ActivationFunctionType.Copy

            def cpass(fsb, fpsp, sre, sim_n, sim_, re_in, im_in, stacked=False):
                """complex pass: re_out = sre@re + sim_n@im ; im_out = sim_@re + sre@im
                (sim_n = negated imag matrix). Round-copies out of PSUM, re on
                DVE and im on ScalarE so the two copies run concurrently.
                stacked=True returns one (128,512) bf16 tile [re | im] for A2A
                staging (bf16 wire format halves the collective bytes)."""
                ps_re = fpsp.tile([N, 512], FP, tag="psre")
                ps_im = fpsp.tile([N, 512], FP, tag="psim")
                nc.tensor.matmul(ps_re[:], lhsT=sre, rhs=re_in[:], start=True, stop=False)
                nc.tensor.matmul(ps_re[:], lhsT=sim_n, rhs=im_in[:], start=False, stop=True)
                nc.tensor.matmul(ps_im[:], lhsT=sim_, rhs=re_in[:], start=True, stop=False)
                nc.tensor.matmul(ps_im[:], lhsT=sre, rhs=im_in[:], start=False, stop=True)
                if stacked:
                    o = fsb.tile([128, 512], BF, tag="fstk")
                    nc.vector.tensor_copy(o[0:N, :], ps_re[:])
                    nc.scalar.activation(o[N:128, :], ps_im[:], Copy)
                    return o
                o_re = fsb.tile([N, 512], FR, tag="fre")
                o_im = fsb.tile([N, 512], FR, tag="fim")
                nc.vector.tensor_copy(o_re[:], ps_re[:])
                nc.scalar.activation(o_im[:], ps_im[:], Copy)
                return o_re, o_im

            dma_engines = (nc.sync, nc.scalar, nc.gpsimd)

            def tstage(fsb, tps, re_in, im_in, strided_out):
                """transpose stage. Input [p | blk*64+q] (blk outer, q inner 64).
                strided_out=False: out[q | blk*64+p]  (contiguous 64-col writes)
                strided_out=True:  out[q | p*8+blk]   (stride-8 writes, 8 blocks)
                All 4 transposed blocks land in one PSUM tile so the
                PSUM->SBUF moves are 2 big strided copies instead of 8.
                """
                o_re = fsb.tile([N, 512], FR, tag="tre")
                o_im = fsb.tile([N, 512], FR, tag="tim")
                for (src, dst, eng) in ((re_in, o_re, 0), (im_in, o_im, 1)):
                    pt = tps.tile([128, 4, N], FR, tag="pt")
                    for t in range(4):
                        nc.tensor.transpose(pt[:, t, :], src[:, t * 128:(t + 1) * 128],
                                            ident)
                    # block blk = 2t+h: dst cols from pt[64h:64h+64, t, :]
                    # re-copies on DVE, im-copies on ScalarE (concurrent)
                    if strided_out:
                        dstv = dst.rearrange("w (i t h) -> w t i h", i=N, t=4, h=2)
                        for h in range(2):
                            if eng == 0:
                                nc.vector.tensor_copy(dstv[:, :, :, h],
                                                      pt[h * N:(h + 1) * N])
                            else:
                                nc.scalar.activation(dstv[:, :, :, h],
                                                     pt[h * N:(h + 1) * N], Copy)
                    else:
                        dstv = dst.rearrange("w (t h i) -> w t h i", t=4, h=2, i=N)
                        for h in range(2):
                            if eng == 0:
                                nc.vector.tensor_copy(dstv[:, :, h],
                                                      pt[h * N:(h + 1) * N])
                            else:
                                nc.scalar.activation(dstv[:, :, h],
                                                     pt[h * N:(h + 1) * N], Copy)
                return o_re, o_im

            fsb_pool = tc.tile_pool(name="fft", bufs=2)
            fsb = fsb_pool.__enter__()
            # forward z was computed in phase 1; round-copy out
            f_re = fsb.tile([N, 512], FR, tag="fre")
            f_im = fsb.tile([N, 512], FR, tag="fim")
            nc.vector.tensor_copy(f_re[:], accps[0:N, :])
            nc.vector.tensor_copy(f_im[:], accps[N:128, :])
            acc_pool.__exit__(None, None, None)
            with tc.tile_pool(name="fps", bufs=2, space="PSUM") as fps, \
                 tc.tile_pool(name="ps6p", bufs=1, space="PSUM") as ps6p, \
                 tc.tile_pool(name="tps", bufs=3, space="PSUM") as tps:
                tap("dbg_f1re", f_re); tap("dbg_f1im", f_im)
                t_re, t_im = tstage(fsb, tps, f_re, f_im, True)   # [y | kz*8+xl]
                tap("dbg_t1re", t_re); tap("dbg_t1im", t_im)
                f2s = cpass(fsb, fps, ArT, AiTn, AiT, t_re, t_im,
                            stacked=True)                 # [ky± | kz*8+xl]

                # A2A #1: -> [ky | kzl*64 + x]   (chunk for dest d = cols [64d,64d+64))
                a_in = dram.tile([N_CORES, 2, N, 8, 8], BF, tag="a2a_in")
                a_out = dram.tile([N_CORES, 2, N, 8, 8], BF, tag="a2a_out")
                for dd in range(N_CORES):
                    dma_engines[dd % 3].dma_start(
                        a_in[dd].rearrange("q p kl xl -> (q p) kl xl"),
                        f2s[:, dd * N:(dd + 1) * N].rearrange(
                            "p (kl xl) -> p kl xl", kl=8, xl=8))
                nc.gpsimd.collective_compute(
                    "AllToAll", mybir.AluOpType.bypass,
                    replica_groups=[list(range(N_CORES))],
                    ins=[a_in.opt()], outs=[a_out.opt()])
                # recv contiguously (cheap descriptors), then DVE-permute
                # [p | s*64+kl*8+xl] -> [p | kl*64+s*8+xl]
                g_raw = fsb.tile([128, 512], BF, tag="graw")
                for ss in range(N_CORES):
                    dma_engines[ss % 3].dma_start(
                        g_raw[:, ss * N:(ss + 1) * N].rearrange(
                            "p (kl xl) -> p kl xl", kl=8, xl=8),
                        a_out[ss].rearrange("q p kl xl -> (q p) kl xl"))
                g_re = fsb.tile([N, 512], FR, tag="fre")
                g_im = fsb.tile([N, 512], FR, tag="fim")
                nc.vector.tensor_copy(
                    g_re.rearrange("p (kl s xl) -> p s kl xl", kl=8, s=8, xl=8),
                    g_raw[0:N, :].rearrange("p (s kl xl) -> p s kl xl", s=8, kl=8, xl=8))
                nc.scalar.activation(
                    g_im.rearrange("p (kl s xl) -> p s kl xl", kl=8, s=8, xl=8),
                    g_raw[N:128, :].rearrange("p (s kl xl) -> p s kl xl", s=8, kl=8, xl=8),
                    Copy)

                tap("dbg_g2re", g_re); tap("dbg_g2im", g_im)
                t2_re, t2_im = tstage(fsb, tps, g_re, g_im, False)  # [x | kzl*64+ky]
                tap("dbg_t2re", t2_re); tap("dbg_t2im", t2_im)
                # P3 forward x, then filter fused into the PSUM->SBUF copy
                ps3_re = fps.tile([N, 512], FP, tag="psre")
                ps3_im = fps.tile([N, 512], FP, tag="psim")
                nc.tensor.matmul(ps3_re[:], lhsT=ArT, rhs=t2_re[:], start=True, stop=False)
                nc.tensor.matmul(ps3_re[:], lhsT=AiTn, rhs=t2_im[:], start=False, stop=True)
                nc.tensor.matmul(ps3_im[:], lhsT=AiT, rhs=t2_re[:], start=True, stop=False)
                nc.tensor.matmul(ps3_im[:], lhsT=ArT, rhs=t2_im[:], start=False, stop=True)
                f3_re = fsb.tile([N, 512], FR, tag="fre")
                f3_im = fsb.tile([N, 512], FR, tag="fim")
                nc.vector.tensor_tensor(f3_re[:], ps3_re[:], ham_sb[:], op=mybir.AluOpType.mult)
                nc.vector.tensor_tensor(f3_im[:], ps3_im[:], ham_sb[:], op=mybir.AluOpType.mult)

                tap("dbg_f3re", f3_re); tap("dbg_f3im", f3_im)
                g4_re, g4_im = cpass(fsb, fps, BrT, BiTn, BiT, f3_re, f3_im)  # [x | kzl*64+ky]
                t3_re, t3_im = tstage(fsb, tps, g4_re, g4_im, True)  # [ky | x*8+kzl]
                f5s = cpass(fsb, fps, BrT, BiTn, BiT, t3_re, t3_im, stacked=True)  # [y± | x*8+kzl]

                # A2A #2: -> [y | xl*64 + kz]   (chunk for dest d = cols [64d,64d+64))
                a2_in = dram.tile([N_CORES, 2, N, 8, 8], BF, tag="a2a2_in")
                a2_out = dram.tile([N_CORES, 2, N, 8, 8], BF, tag="a2a2_out")
                for dd in range(N_CORES):
                    dma_engines[dd % 3].dma_start(
                        a2_in[dd].rearrange("q p xl kl -> (q p) xl kl"),
                        f5s[:, dd * N:(dd + 1) * N].rearrange("p (xl kl) -> p xl kl", xl=8, kl=8))
                nc.gpsimd.collective_compute(
                    "AllToAll", mybir.AluOpType.bypass,
                    replica_groups=[list(range(N_CORES))],
                    ins=[a2_in.opt()], outs=[a2_out.opt()])
                # recv contiguously, then DVE-permute [p | s*64+xl*8+kl] -> [p | xl*64+s*8+kl]
                g5_raw = fsb.tile([128, 512], BF, tag="graw")
                for ss in range(N_CORES):
                    dma_engines[ss % 3].dma_start(
                        g5_raw[:, ss * N:(ss + 1) * N].rearrange("p (xl kl) -> p xl kl", xl=8, kl=8),
                        a2_out[ss].rearrange("q p xl kl -> (q p) xl kl"))
                g5_re = fsb.tile([N, 512], FR, tag="fre")
                g5_im = fsb.tile([N, 512], FR, tag="fim")
                nc.vector.tensor_copy(
                    g5_re.rearrange("p (xl s kl) -> p s xl kl", xl=8, s=8, kl=8),
                    g5_raw[0:N, :].rearrange("p (s xl kl) -> p s xl kl", s=8, xl=8, kl=8))
                nc.scalar.activation(
                    g5_im.rearrange("p (xl s kl) -> p s xl kl", xl=8, s=8, kl=8),
                    g5_raw[N:128, :].rearrange("p (s xl kl) -> p s xl kl", s=8, xl=8, kl=8),
                    Copy)

                tap("dbg_g5re", g5_re); tap("dbg_g5im", g5_im)
                t4_re, t4_im = tstage(fsb, tps, g5_re, g5_im, False)  # [kz | xl*64+y]
                # P6: inverse z, real part only
                ps6 = ps6p.tile([N, 512], FP, tag="ps6")
                nc.tensor.matmul(ps6[:], lhsT=BrT, rhs=t4_re[:], start=True, stop=False)
                nc.tensor.matmul(ps6[:], lhsT=BiTn, rhs=t4_im[:], start=False, stop=True)
                out_sb = fsb.tile([N, 512], FP, tag="osb")
                nc.vector.tensor_copy(out_sb[:], ps6[:])
                nc.sync.dma_start(out[:], out_sb[:])
            fsb_pool.__exit__(None, None, None)

    nc.compile()
    return nc


def _get_compiled():
    global _COMPILED
    if _COMPILED is None:
        _COMPILED = _build()
    return _COMPILED


def _host_inputs(X, aw, bw, real_grid_flat, hamming):
    X = np.asarray(X, np.float32)
    aw = np.asarray(aw, np.float32)
    bw = np.asarray(bw, np.float32)
    grid = np.asarray(real_grid_flat, np.float32)
    hamming = np.asarray(hamming, np.float32)

    arr = grid.reshape(N, N, N, 3)                       # [x, y, z, 3]
    xs = arr[:, 0, 0, 0]                                 # (64,)
    ys = arr[0, :, 0, 1]
    zs = arr[0, 0, :, 2]
    d2y = (ys[None, :] - X[:, 1:2]) ** 2                 # (128, 64)
    d2z = (zs[None, :] - X[:, 2:3]) ** 2                 # (128, 64)

    lnaw3 = (np.log(np.maximum(aw, 1e-38)) / 3.0).astype(np.float32)
    bwln = np.concatenate([bw, lnaw3], 1).astype(np.float32)    # (128, 12)

    F = np.fft.fft(np.eye(N), axis=0, norm='ortho')
    IF = np.fft.ifft(np.eye(N), axis=0, norm='ortho')
    perm = (np.arange(N) + N // 2) % N
    Am = F[:, perm]
    Bm = IF[perm, :]
    Ar, Ai = Am.real.astype(np.float32), Am.imag.astype(np.float32)
    Br, Bi = Bm.real.astype(np.float32), Bm.imag.astype(np.float32)
    Az2T = np.concatenate([Ar.T, Ai.T], 1)               # (64, 128)
    mats = np.concatenate(
        [Az2T, Ar.T, Ai.T, -Ai.T, Br.T, Bi.T, -Bi.T,
         np.eye(N, dtype=np.float32)], 1)                # (64, 576)

    Hfull = np.fft.ifftshift(hamming)                    # [kx, ky, kz]

    in_maps = []
    for c in range(N_CORES):
        d2x = (xs[None, 8 * c:8 * (c + 1)] - X[:, 0:1]) ** 2    # (128, 8)
        d2all = np.concatenate([d2x, d2y, d2z], 1).astype(np.float32)
        Hc = np.ascontiguousarray(
            np.transpose(Hfull[:, :, 8 * c:8 * (c + 1)], (0, 2, 1))).reshape(N, 512)
        in_maps.append({"d2all": d2all, "bwln": bwln,
                        "mats": mats, "ham": Hc})
    return in_maps


def kernel(X, aw, bw, real_grid_flat, hamming):
    global LAST_EXEC_NS, LAST_RESULTS
    in_maps = _host_inputs(X, aw, bw, real_grid_flat, hamming)
    nc = _get_compiled()

    trace = bool(os.environ.get("BASS_TRACE"))
    res = run_bass_kernel_spmd(nc, in_maps, core_ids=list(range(N_CORES)),
                               trace=trace)
    LAST_EXEC_NS = res.exec_time_ns
    global LAST_RESULTS
    LAST_RESULTS = res.results

    full = np.empty((N, N, N), np.float32)               # [z, x, y]
    for c in range(N_CORES):
        full[:, 8 * c:8 * (c + 1), :] = res.results[c]["out"].reshape(N, 8, N)
    o = np.transpose(full, (1, 2, 0))                    # [x, y, z]
    o = (o - o.mean()) / (o.std() + 1e-8)
    return o.astype(np.float32)

